# revision 2
# baseline (speedup 1.0000x reference)
"""2-layer GAT (PyG GATConv semantics) on 8 Trainium2 NeuronCores.

Strategy (graph/data parallel, per sharding hint):
  - Nodes partitioned into 8 contiguous ranges (6250 per core).
  - Each core computes h = x @ W plus per-node attention scalars (a_src, a_dst)
    for its own slice, packs them into table rows [h(128) | a_s(4) | a_d(4)],
    and an AllGather replicates the full 50000x136 table to every core.
  - Edges (with self loops) are sorted by destination and assigned to the core
    owning the destination; within a core they are grouped into 49 blocks of
    128 destination slots, each padded to a uniform number of 128-edge chunks.
  - Per chunk, source rows are fetched with an indirect DMA gather (one row per
    partition); the per-destination softmax + weighted aggregation is done with
    one-hot matmuls into PSUM (segment softmax without max subtraction -- exact
    up to fp rounding since alphas are O(10)).
  - Layer 2 repeats the same edge program against the layer-2 table.

Raw Bass (no Tile -- the Tile drain is incompatible with this toolchain),
hand-scheduled with per-engine semaphore counts, 3-deep block pipelining.
One-hot matrices and messages are bf16 (values exact / tolerance-safe).
"""
import sys

sys.path.insert(0, "/opt/trn_rl_repo")

import math
from contextlib import ExitStack

import numpy as np

import concourse.bass as bass
import concourse.mybir as mybir

F32 = mybir.dt.float32
BF16 = mybir.dt.bfloat16
I32 = mybir.dt.int32
P = 128

# problem constants (hardcoded per contract)
N_NODES = 50000
N_EDGES = 800000
IN_DIM = 256
OUT_DIM = 32
HEADS = 4
HC = OUT_DIM * HEADS  # 128
NEG_SLOPE = 0.2
NCORES = 8
TW = HC + 2 * HEADS  # table row width: h | a_src | a_dst = 136
SENTINEL = 320.0     # pad-edge dst_local (bf16-exact, >= 128 so one-hot is 0)
GS = 4               # gather/message/meta pipeline depth
CHUNKS1 = (0, 2048, 4096, 6144)   # L1 table AllGather chunk row starts
CHUNKS2 = (0, 2048, 4096, 5888)   # L2 table AllGather chunk row starts
CC_OVERLAP = True
MONO_CC = False
ABLATE_GATHER = False


# ----------------------------------------------------------------------------
# Bass program builder
# ----------------------------------------------------------------------------

def build_gat(Nc, KCs, in_dim=IN_DIM):
    NB = math.ceil(Nc / 128)  # destination blocks == node tiles
    NT = NB
    KS = in_dim // P
    assert in_dim % P == 0
    assert len(KCs) == NB
    KC = max(KCs)  # array-sizing chunk count (incl. self chunk 0)

    nc = bass.Bass()

    # ---- DRAM parameters ----
    xT = nc.declare_dram_parameter("xT", [in_dim, Nc], F32, isOutput=False)
    W1 = nc.declare_dram_parameter("W1", [in_dim, HC], F32, isOutput=False)
    W2 = nc.declare_dram_parameter("W2", [HC, HC], F32, isOutput=False)
    asrc1 = nc.declare_dram_parameter("asrc1", [P, HC], F32, isOutput=False)
    adst1 = nc.declare_dram_parameter("adst1", [P, HC], F32, isOutput=False)
    asrc2 = nc.declare_dram_parameter("asrc2", [P, HC], F32, isOutput=False)
    adst2 = nc.declare_dram_parameter("adst2", [P, HC], F32, isOutput=False)
    bb1 = nc.declare_dram_parameter("bb1", [P, HC], F32, isOutput=False)
    bb2 = nc.declare_dram_parameter("bb2", [P, HC], F32, isOutput=False)
    iob = nc.declare_dram_parameter("iob", [P, P], BF16, isOutput=False)
    idn = nc.declare_dram_parameter("idn", [P, P], F32, isOutput=False)
    idb = nc.declare_dram_parameter("idb", [P, P], BF16, isOutput=False)
    kap1 = nc.declare_dram_parameter("kap1", [KC, KC * HEADS], F32, isOutput=False)
    kap2 = nc.declare_dram_parameter("kap2", [KC, KC * HEADS], F32, isOutput=False)
    midx1 = nc.declare_dram_parameter("midx1", [NB, P, KC], I32, isOutput=False)
    midx2 = nc.declare_dram_parameter("midx2", [NB, P, KC], I32, isOutput=False)
    mdst = nc.declare_dram_parameter("mdst", [NB, P, KC], F32, isOutput=False)
    mea = nc.declare_dram_parameter("mea", [NB, KC, P], F32, isOutput=False)
    out_p = nc.declare_dram_parameter("out", [Nc, HC], F32, isOutput=True)

    # ---- internal DRAM ----
    hA1s = nc.dram_tensor("hA1s", [Nc, TW], F32)
    hA2s = nc.dram_tensor("hA2s", [Nc, TW], F32)
    hA1f = nc.dram_tensor("hA1f", [Nc * NCORES, TW], F32, addr_space="Shared")
    hA2f = nc.dram_tensor("hA2f", [Nc * NCORES, TW], F32, addr_space="Shared")

    ctx = ExitStack()
    sb = lambda name, shape, dt=F32: ctx.enter_context(
        nc.sbuf_tensor(name, shape, dt))
    ps = lambda name, shape, dt=F32: ctx.enter_context(
        nc.psum_tensor(name, shape, dt))

    # ---- SBUF ----
    W1_sb = sb("W1_sb", [P, KS, P])
    W2_sb = sb("W2_sb", [P, HC])
    asrc1_sb = sb("asrc1_sb", [P, HC]); adst1_sb = sb("adst1_sb", [P, HC])
    asrc2_sb = sb("asrc2_sb", [P, HC]); adst2_sb = sb("adst2_sb", [P, HC])
    bb1_sb = sb("bb1_sb", [P, HC]); bb2_sb = sb("bb2_sb", [P, HC])
    iob_sb = sb("iob_sb", [P, P], BF16)
    idn_sb = sb("idn_sb", [P, P]); idb_sb = sb("idb_sb", [P, P], BF16)
    kap1_sb = sb("kap1_sb", [KC, KC * HEADS])
    kap2_sb = sb("kap2_sb", [KC, KC * HEADS])
    xk_sb = sb("xk_sb", [P, 2, KS, 4 * P])
    htile_sb = sb("htile_sb", [P, 2, 4, TW])
    tmp_sb = sb("tmp_sb", [P, 4, HC])
    h1_sb = sb("h1_sb", [P, NT * P])
    h1T_sb = sb("h1T_sb", [P, 2, P])
    g_sb = sb("g_sb", [P, GS, KC, TW])
    msg_sb = sb("msg_sb", [P, GS, KC, TW], BF16)
    M_sb = sb("M_sb", [P, 2, KC, P], BF16)
    Mt_sb = sb("Mt_sb", [P, 2, KC, P], BF16)
    idx_sb = sb("idx_sb", [P, GS, KC], I32)
    dst_sb = sb("dst_sb", [P, GS, KC])
    eaT_sb = sb("eaT_sb", [KC, GS, P])
    adbb_sb = sb("adbb_sb", [P, GS, HEADS], BF16)
    alpha_sb = sb("alpha_sb", [P, 2, KC, HEADS])
    alph2_sb = sb("alph2_sb", [P, KC, HEADS])
    asg_sb = sb("asg_sb", [P, 2, KC, HEADS])
    recip_sb = sb("recip_sb", [P, HEADS])
    outsb = sb("outsb", [P, 2, HC])

    # ---- PSUM (8 banks) ----
    ph = ps("ph0", [P, 4 * P])                    # h matmuls (A, D), 1 bank
    ptf = ps("ptf", [P, P])                       # f32 transposes (phase D)
    ptb = [ps("ptb0", [P, P], BF16), ps("ptb1", [P, P], BF16)]  # M transposes
    pa = [ps("pa0", [P, KC * HEADS]), ps("pa1", [P, KC * HEADS])]
    po = [ps("po0", [P, HC + HEADS]), ps("po1", [P, HC + HEADS])]

    # ---- semaphores / plan ----
    sem_names = ["s_sp", "s_pool", "s_pe", "s_dve", "s_act", "s_cc",
                 "s_spr", "s_plr"]
    sems = {n: ctx.enter_context(nc.semaphore(n)) for n in sem_names}
    ENG_SEM = {"sp": "s_sp", "pool": "s_pool", "pe": "s_pe", "dve": "s_dve",
               "act": "s_act"}
    RDY_SEM = {"sp": "s_spr", "pool": "s_plr"}

    cnt = {n: 0 for n in sem_names}
    items = {e: [] for e in ENG_SEM}
    m = {}

    def em(eng, fn=None, dma=False, cc=False, raw=False, waits=()):
        inc = None
        if fn is not None and not raw:
            if cc:
                inc = ("s_cc", 1)
            elif dma:
                inc = (ENG_SEM[eng], 16)
            else:
                inc = (ENG_SEM[eng], 1)
            cnt[inc[0]] += inc[1]
        items[eng].append((fn, [(s, v) for (s, v) in waits if v > 0], inc))

    def drain(eng):
        em(eng, lambda e: e.drain(), raw=True)

    def flush(eng):
        rs = RDY_SEM[eng]
        em(eng, None, waits=[(ENG_SEM[eng], cnt[ENG_SEM[eng]])])
        em(eng, (lambda rs=rs: (lambda e: e.sem_inc(sems[rs], 1)))(), raw=True)
        cnt[rs] += 1
        return cnt[rs]

    def barrier():
        snap = dict(cnt)
        for e in ENG_SEM:
            em(e, None, waits=[(s, snap[s]) for s in sem_names])

    AL = mybir.AluOpType
    AF = mybir.ActivationFunctionType
    AX = mybir.AxisListType

    # ======================= const loads =======================
    def cdma(dst_ap, src_ap):
        em("sp", (lambda d=dst_ap, s=src_ap: (lambda e: e.dma_start(out=d, in_=s)))(),
           dma=True)

    for k in range(KS):
        cdma(W1_sb[:, k, :], W1[k * P:(k + 1) * P, :])
    cdma(W2_sb[:], W2[:])
    cdma(asrc1_sb[:], asrc1[:]); cdma(adst1_sb[:], adst1[:])
    cdma(asrc2_sb[:], asrc2[:]); cdma(adst2_sb[:], adst2[:])
    cdma(bb1_sb[:], bb1[:]); cdma(bb2_sb[:], bb2[:])
    cdma(iob_sb[:], iob[:]); cdma(idn_sb[:], idn[:]); cdma(idb_sb[:], idb[:])
    cdma(kap1_sb[:], kap1[:]); cdma(kap2_sb[:], kap2[:])
    m["constsConf"] = flush("sp")
    # self-chunk rows beyond nd of the last block are read (harmlessly) --
    # give them defined values once
    em("dve", lambda e: e.memset(g_sb[:], 0.0))

    # ======================= table phase A (batched groups of 4 tiles) ======
    GT = 4                      # tiles per group
    NG = math.ceil(Nc / (GT * P))

    def plan_tableA():
        p = "A"
        for g in range(NG):
            c0 = g * GT * P
            ng = min(GT * P, Nc - c0)
            jt = math.ceil(ng / P)          # tiles in this group
            s = g % 2
            w = [("s_pe", m.get((p, "mm", g - 2), 0))]
            for k in range(KS):
                em("sp", (lambda s=s, k=k, c0=c0, ng=ng: (lambda e: e.dma_start(
                    out=xk_sb[:, s, k, 0:ng],
                    in_=xT[k * P:(k + 1) * P, c0:c0 + ng])))(),
                   dma=True, waits=w if k == 0 else ())
            m[(p, "conf", g)] = flush("sp")
            if g >= 1:
                g1 = g - 1
                c1 = g1 * GT * P
                ng1 = min(GT * P, Nc - c1)
                jt1 = math.ceil(ng1 / P)
                if ng1 == GT * P:
                    dst_ap = hA1s[c1:c1 + ng1, :].rearrange(
                        "(j q) c -> q j c", q=P)
                    src_ap = htile_sb[:, 1 - s, :, :]
                else:
                    dst_ap = hA1s[c1:c1 + ng1, :]
                    src_ap = htile_sb[0:ng1, 1 - s, 0, :]
                em("sp", (lambda d=dst_ap, sr=src_ap: (lambda e: e.dma_start(
                    out=d, in_=sr)))(),
                   dma=True, waits=[("s_dve", m[(p, "ops", g1)])])
            w = [("s_spr", m[(p, "conf", g)]),
                 ("s_dve", m.get((p, "ops", g - 1), 0))]
            first = True
            for j in range(jt):
                nt = min(P, ng - j * P)
                for k in range(KS):
                    em("pe", (lambda s=s, k=k, j=j, nt=nt, c0=c0: (lambda e: e.matmul(
                        out=ph[0:nt, j * P:j * P + P],
                        lhsT=xk_sb[:, s, k, j * P:j * P + nt],
                        rhs=W1_sb[:, k, :], start=(k == 0), stop=(k == KS - 1),
                        skip_group_check=True)))(),
                       waits=w if first else ())
                    first = False
            m[(p, "mm", g)] = cnt["s_pe"]
            # DVE: pack table rows
            w = [("s_pe", m[(p, "mm", g)]),
                 ("s_spr", m.get((p, "conf", g), 0) if g >= 2 else 0)]
            if ng == GT * P:
                em("dve", (lambda s=s, jt=jt: (lambda e: e.tensor_copy(
                    out=htile_sb[:, s, 0:jt, 0:HC],
                    in_=ph[:, 0:jt * P].rearrange(
                        "q (j c) -> q j c", j=jt))))(), waits=w)
                m[(p, "copy", g)] = cnt["s_dve"]
                drain("dve")
                em("dve", (lambda s=s, jt=jt: (lambda e: e.tensor_mul(
                    out=tmp_sb[:, 0:jt, :],
                    in0=ph[:, 0:jt * P].rearrange("q (j c) -> q j c", j=jt),
                    in1=asrc1_sb[:, None, :].to_broadcast([P, jt, HC]))))())
                drain("dve")
                em("dve", (lambda s=s, jt=jt: (lambda e: e.tensor_reduce(
                    out=htile_sb[:, s, 0:jt, HC:HC + HEADS],
                    in_=tmp_sb[:, 0:jt, :].rearrange(
                        "q j (h c) -> q j h c", c=OUT_DIM),
                    axis=AX.X, op=AL.add)))())
                drain("dve")
                em("dve", (lambda s=s, jt=jt: (lambda e: e.tensor_mul(
                    out=tmp_sb[:, 0:jt, :],
                    in0=ph[:, 0:jt * P].rearrange("q (j c) -> q j c", j=jt),
                    in1=adst1_sb[:, None, :].to_broadcast([P, jt, HC]))))())
                drain("dve")
                em("dve", (lambda s=s, jt=jt: (lambda e: e.tensor_reduce(
                    out=htile_sb[:, s, 0:jt, HC + HEADS:TW],
                    in_=tmp_sb[:, 0:jt, :].rearrange(
                        "q j (h c) -> q j h c", c=OUT_DIM),
                    axis=AX.X, op=AL.add)))())
            else:
                first = True
                for j in range(jt):
                    nt = min(P, ng - j * P)
                    em("dve", (lambda s=s, j=j, nt=nt: (lambda e: e.tensor_copy(
                        out=htile_sb[0:nt, s, j, 0:HC],
                        in_=ph[0:nt, j * P:j * P + P])))(),
                       waits=w if first else ())
                    first = False
                    if j == 0:
                        m[(p, "copy", g)] = cnt["s_dve"]
                    drain("dve")
                    em("dve", (lambda s=s, j=j, nt=nt: (lambda e: e.tensor_mul(
                        out=tmp_sb[0:nt, 0, :],
                        in0=ph[0:nt, j * P:j * P + P],
                        in1=asrc1_sb[0:nt, :])))())
                    drain("dve")
                    em("dve", (lambda s=s, j=j, nt=nt: (lambda e: e.tensor_reduce(
                        out=htile_sb[0:nt, s, j, HC:HC + HEADS],
                        in_=tmp_sb[0:nt, 0, :].rearrange(
                            "q (h c) -> q h c", c=OUT_DIM),
                        axis=AX.X, op=AL.add)))())
                    drain("dve")
                    em("dve", (lambda s=s, j=j, nt=nt: (lambda e: e.tensor_mul(
                        out=tmp_sb[0:nt, 0, :],
                        in0=ph[0:nt, j * P:j * P + P],
                        in1=adst1_sb[0:nt, :])))())
                    drain("dve")
                    em("dve", (lambda s=s, j=j, nt=nt: (lambda e: e.tensor_reduce(
                        out=htile_sb[0:nt, s, j, HC + HEADS:TW],
                        in_=tmp_sb[0:nt, 0, :].rearrange(
                            "q (h c) -> q h c", c=OUT_DIM),
                        axis=AX.X, op=AL.add)))())
            m[(p, "ops", g)] = cnt["s_dve"]
        g = NG - 1
        c0 = g * GT * P
        ng = min(GT * P, Nc - c0)
        if ng == GT * P:
            dst_ap = hA1s[c0:c0 + ng, :].rearrange("(j q) c -> q j c", q=P)
            src_ap = htile_sb[:, g % 2, :, :]
        else:
            jt = math.ceil(ng / P)
            dst_ap = hA1s[c0:c0 + ng, :]
            src_ap = None  # handled below per-tile for ragged tail
        if src_ap is not None:
            em("sp", (lambda d=dst_ap, sr=src_ap: (lambda e: e.dma_start(
                out=d, in_=sr)))(),
               dma=True, waits=[("s_dve", m[(p, "ops", g)])])
        else:
            # ragged: store tile by tile
            w = [("s_dve", m[(p, "ops", g)])]
            for j in range(math.ceil(ng / P)):
                nt = min(P, ng - j * P)
                em("sp", (lambda s=g % 2, j=j, nt=nt, c0=c0: (lambda e: e.dma_start(
                    out=hA1s[c0 + j * P:c0 + j * P + nt, :],
                    in_=htile_sb[0:nt, s, j, :])))(),
                   dma=True, waits=w if j == 0 else ())
        m[(p, "allConf")] = flush("sp")

    # --- layer-2 table tile (merged into layer-1 edge pipeline) ---
    def plan_tableD_tile(t):
        p = "D"
        s = t % 2
        em("pe", (lambda t=t: (lambda e: e.transpose(
            out=ptf[:], in_=h1_sb[:, t * P:(t + 1) * P],
            identity=idn_sb[:])))(),
           waits=[("s_dve", m[("C", "epi", t)]),
                  ("s_dve", m.get((p, "c1", t - 1), 0))])
        m[(p, "T", t)] = cnt["s_pe"]
        em("dve", (lambda s=s: (lambda e: e.tensor_copy(
            out=h1T_sb[:, s, :], in_=ptf[:])))(),
           waits=[("s_pe", m[(p, "T", t)]),
                  ("s_pe", m.get((p, "mm", t - 2), 0))])
        m[(p, "c1", t)] = cnt["s_dve"]
        em("pe", (lambda s=s: (lambda e: e.matmul(
            out=ph[:, 0:HC], lhsT=h1T_sb[:, s, :], rhs=W2_sb[:],
            start=True, stop=True)))(),
           waits=[("s_dve", m[(p, "c1", t)]),
                  ("s_dve", m.get((p, "ops", t - 1), 0))])
        m[(p, "mm", t)] = cnt["s_pe"]
        nt = min(P, Nc - t * P)
        w = [("s_pe", m[(p, "mm", t)]),
             ("s_spr", m.get((p, "stconf", t - 2), 0))]
        em("dve", (lambda s=s: (lambda e: e.tensor_copy(
            out=htile_sb[:, s, 0, 0:HC], in_=ph[:, 0:HC])))(), waits=w)
        m[(p, "copy", t)] = cnt["s_dve"]
        drain("dve")
        em("dve", (lambda s=s: (lambda e: e.tensor_mul(
            out=tmp_sb[:, 0, :], in0=ph[:, 0:HC],
            in1=asrc2_sb[:])))())
        drain("dve")
        em("dve", (lambda s=s: (lambda e: e.tensor_reduce(
            out=htile_sb[:, s, 0, HC:HC + HEADS],
            in_=tmp_sb[:, 0, :].rearrange("q (h c) -> q h c", c=OUT_DIM),
            axis=AX.X, op=AL.add)))())
        drain("dve")
        em("dve", (lambda s=s: (lambda e: e.tensor_mul(
            out=tmp_sb[:, 0, :], in0=ph[:, 0:HC],
            in1=adst2_sb[:])))())
        drain("dve")
        em("dve", (lambda s=s: (lambda e: e.tensor_reduce(
            out=htile_sb[:, s, 0, HC + HEADS:TW],
            in_=tmp_sb[:, 0, :].rearrange("q (h c) -> q h c", c=OUT_DIM),
            axis=AX.X, op=AL.add)))())
        m[(p, "ops", t)] = cnt["s_dve"]

    # ======================= edge phase (C: layer1, F: layer2) ===============
    # Software-pipelined: block b's gathers are issued in loop iter b, its
    # compute consumption happens in iter b+1, and the Pool confirmation flush
    # runs only every second block so the DMA ring never fully drains per
    # block. Chunk 0 of each block is the self-loop chunk (regular DMA,
    # identity one-hot, doubles as a_dst source + denominator guarantee).
    def plan_edges(layer):
        p = "C" if layer == 1 else "F"
        hAf = hA1f if layer == 1 else hA2f
        hAs = hA1s if layer == 1 else hA2s
        midx = midx1 if layer == 1 else midx2
        kap_sb = kap1_sb if layer == 1 else kap2_sb
        bias_sb = bb1_sb if layer == 1 else bb2_sb
        gN = {}

        def plan_meta(b):
            nd = min(P, Nc - b * P)
            s = b % GS
            w = [("s_plr", gN.get(b - GS, 0)),
                 ("s_pe", m.get((p, "scat", b - GS), 0))]
            em("sp", (lambda s=s, b=b: (lambda e: e.dma_start(
                out=idx_sb[:, s, :], in_=midx[b])))(), dma=True, waits=w)
            em("sp", (lambda s=s, b=b: (lambda e: e.dma_start(
                out=dst_sb[:, s, :], in_=mdst[b])))(), dma=True)
            em("sp", (lambda s=s, b=b: (lambda e: e.dma_start(
                out=eaT_sb[:, s, :], in_=mea[b])))(), dma=True)
            em("sp", (lambda s=s, b=b, nd=nd, hAs=hAs: (lambda e: e.dma_start(
                out=g_sb[0:nd, s, 0, :],
                in_=hAs[b * P:b * P + nd, :])))(), dma=True)
            if layer == 1 and b >= 2:
                t1 = b - 2
                nt1 = min(P, Nc - t1 * P)
                em("sp", (lambda s=t1 % 2, t1=t1, nt1=nt1: (lambda e: e.dma_start(
                    out=hA2s[t1 * P:t1 * P + nt1, :],
                    in_=htile_sb[0:nt1, s, 0, :])))(),
                   dma=True, waits=[("s_dve", m[("D", "ops", t1)])])
            if layer == 2 and b >= 2:
                b1 = b - 2
                nd1 = min(P, Nc - b1 * P)
                em("sp", (lambda sp=b1 % 2, b1=b1, nd1=nd1: (lambda e: e.dma_start(
                    out=out_p[b1 * P:b1 * P + nd1, :],
                    in_=outsb[0:nd1, sp, :])))(),
                   dma=True, waits=[("s_dve", m[(p, "epi", b1)])])
            m[(p, "conf", b)] = flush("sp")
            if layer == 1 and b >= 2:
                m[("D", "stconf", b - 2)] = m[(p, "conf", b)]

        def plan_gather(b):
            KCb = KCs[b]
            s = b % GS
            w = [("s_spr", m[(p, "conf", b)]),
                 ("s_dve", m.get((p, "msgs", b - GS), 0))]
            for c in range(1, KCb):
                if ABLATE_GATHER:
                    break
                em("pool", (lambda s=s, c=c, hAf=hAf: (lambda e: e.indirect_dma_start(
                    out=g_sb[:, s, c, :], out_offset=None, in_=hAf[:],
                    in_offset=bass.IndirectOffsetOnAxis(
                        ap=idx_sb[:, s, c:c + 1], axis=0))))(),
                   dma=True, waits=w if c == 1 else ())
            if b % 2 == 1 or b == NB - 1:
                v = flush("pool")
                for bb in range(max(0, b - 1), b + 1):
                    gN[bb] = v

        def plan_consume(b):
            KCb = KCs[b]
            s = b % GS
            s2 = b % 2
            # DVE: adb cast + one-hot builds
            w = [("s_spr", m[(p, "conf", b)]),
                 ("s_pe", m.get((p, "scat", b - 2), 0))]
            em("dve", (lambda s=s: (lambda e: e.tensor_copy(
                out=adbb_sb[:, s, :], in_=g_sb[:, s, 0, HC + HEADS:TW])))(),
               waits=w)
            for c in range(1, KCb):
                em("dve", (lambda s=s, s2=s2, c=c: (lambda e: e.tensor_scalar(
                    out=M_sb[:, s2, c, :], in0=iob_sb[:],
                    scalar1=dst_sb[:, s, c:c + 1], scalar2=None,
                    op0=AL.is_equal)))())
                m[(p, "Mb", b, c)] = cnt["s_dve"]
            for c in range(1, KCb):
                em("pe", (lambda s2=s2, c=c: (lambda e: e.transpose(
                    out=ptb[c % 2][:], in_=M_sb[:, s2, c, :],
                    identity=idb_sb[:])))(),
                   waits=[("s_dve", m[(p, "Mb", b, c)]),
                          ("s_dve", m.get((p, "Mtc", b, c - 2),
                                          m.get((p, "MtcPrev", b), 0)))])
                m[(p, "T", b, c)] = cnt["s_pe"]
                em("dve", (lambda s2=s2, c=c: (lambda e: e.tensor_copy(
                    out=Mt_sb[:, s2, c, :], in_=ptb[c % 2][:])))(),
                   waits=[("s_pe", m[(p, "T", b, c)])])
                m[(p, "Mtc", b, c)] = cnt["s_dve"]
            m[(p, "MtcPrev", b + 1)] = m[(p, "Mtc", b, KCb - 1)]
            # PE: alpha accumulation
            em("pe", (lambda s=s, s2=s2, KCb=KCb, kap_sb=kap_sb: (lambda e: e.matmul(
                out=pa[s2][:, 0:KCb * HEADS],
                lhsT=eaT_sb[0:KCb, s, :], rhs=kap_sb[0:KCb, 0:KCb * HEADS],
                start=True, stop=False, skip_group_check=True)))(),
               waits=[("s_spr", m[(p, "conf", b)]),
                      ("s_dve", m.get((p, "alpha", b - 2), 0))])
            em("pe", (lambda s=s, s2=s2: (lambda e: e.matmul(
                out=pa[s2][:, 0:HEADS], lhsT=idb_sb[:], rhs=adbb_sb[:, s, :],
                start=False, stop=False, skip_group_check=True)))(),
               waits=[("s_dve", m[(p, "MtcPrev", b + 1)])])
            for c in range(1, KCb):
                em("pe", (lambda s=s, s2=s2, c=c, KCb=KCb: (lambda e: e.matmul(
                    out=pa[s2][:, c * HEADS:(c + 1) * HEADS],
                    lhsT=Mt_sb[:, s2, c, :], rhs=adbb_sb[:, s, :],
                    start=False, stop=(c == KCb - 1), skip_group_check=True)))())
            m[(p, "admm", b)] = cnt["s_pe"]
            # DVE: alpha + leaky relu
            em("dve", (lambda s=s, s2=s2, KCb=KCb: (lambda e: e.tensor_add(
                out=alpha_sb[:, s2, 0:KCb, :],
                in0=g_sb[:, s, 0:KCb, HC:HC + HEADS],
                in1=pa[s2][:, 0:KCb * HEADS].rearrange(
                    "p (k h) -> p k h", h=HEADS))))(),
               waits=[("s_pe", m[(p, "admm", b)]),
                      ("s_plr", gN[b])])
            drain("dve")
            em("dve", (lambda s2=s2, KCb=KCb: (lambda e: e.tensor_scalar(
                out=alph2_sb[:, 0:KCb, :], in0=alpha_sb[:, s2, 0:KCb, :],
                scalar1=NEG_SLOPE, scalar2=None, op0=AL.mult)))())
            drain("dve")
            em("dve", (lambda s2=s2, KCb=KCb: (lambda e: e.tensor_tensor(
                out=alpha_sb[:, s2, 0:KCb, :], in0=alpha_sb[:, s2, 0:KCb, :],
                in1=alph2_sb[:, 0:KCb, :], op=AL.max)))())
            m[(p, "alpha", b)] = cnt["s_dve"]
            # ACT: exp
            em("act", (lambda s=s, s2=s2, KCb=KCb: (lambda e: e.activation(
                out=msg_sb[:, s, 0:KCb, HC:HC + HEADS],
                in_=alpha_sb[:, s2, 0:KCb, :], func=AF.Exp)))(),
               waits=[("s_dve", m[(p, "alpha", b)]),
                      ("s_pe", m.get((p, "scat", b - GS), 0))])
            m[(p, "exp", b)] = cnt["s_act"]
            # DVE: messages
            em("dve", (lambda s=s, KCb=KCb: (lambda e: e.tensor_mul(
                out=msg_sb[:, s, 0:KCb, 0:HC].rearrange(
                    "p k (h c) -> p k h c", c=OUT_DIM),
                in0=g_sb[:, s, 0:KCb, 0:HC].rearrange(
                    "p k (h c) -> p k h c", c=OUT_DIM),
                in1=msg_sb[:, s, 0:KCb, HC:HC + HEADS][:, :, :, None].to_broadcast(
                    [P, KCb, HEADS, OUT_DIM]))))(),
               waits=[("s_act", m[(p, "exp", b)])])
            m[(p, "msgs", b)] = cnt["s_dve"]
            # PE: scatter
            w = [("s_dve", m[(p, "msgs", b)]),
                 ("s_dve", m.get((p, "epi", b - 2), 0))]
            em("pe", (lambda s=s, s2=s2: (lambda e: e.matmul(
                out=po[s2][:], lhsT=idb_sb[:],
                rhs=msg_sb[:, s, 0, 0:HC + HEADS],
                start=True, stop=False)))(), waits=w)
            for c in range(1, KCb):
                em("pe", (lambda s=s, s2=s2, c=c, KCb=KCb: (lambda e: e.matmul(
                    out=po[s2][:], lhsT=M_sb[:, s2, c, :],
                    rhs=msg_sb[:, s, c, 0:HC + HEADS],
                    start=False, stop=(c == KCb - 1))))())
            m[(p, "scat", b)] = cnt["s_pe"]
            # DVE: epilogue
            w = [("s_pe", m[(p, "scat", b)])]
            if layer == 2:
                w.append(("s_spr", m[(p, "conf", b)]))
            em("dve", (lambda s2=s2: (lambda e: e.reciprocal(
                out=recip_sb[:], in_=po[s2][:, HC:HC + HEADS])))(), waits=w)
            if layer == 1:
                tgt = lambda b=b: h1_sb[:, b * P:(b + 1) * P]
            else:
                tgt = lambda s2=s2: outsb[:, s2, :]
            drain("dve")
            em("dve", (lambda s2=s2, tgt=tgt: (lambda e: e.tensor_mul(
                out=tgt().rearrange("p (h c) -> p h c", c=OUT_DIM),
                in0=po[s2][:, 0:HC].rearrange("p (h c) -> p h c", c=OUT_DIM),
                in1=recip_sb[:][:, :, None].to_broadcast(
                    [P, HEADS, OUT_DIM]))))())
            drain("dve")
            em("dve", (lambda tgt=tgt, bias_sb=bias_sb: (lambda e: e.tensor_add(
                out=tgt(), in0=tgt(), in1=bias_sb[:])))())
            m[(p, "epi", b)] = cnt["s_dve"]

        l2cc = {}
        for b in range(NB + 1):
            if b < NB:
                plan_meta(b)
                plan_gather(b)
                if b in l2cc:
                    r0, r1 = l2cc[b]
                    plan_cc_chunk(hA2s, hA2f, r0, r1, m[(p, "conf", b)])
            if b >= 1:
                plan_consume(b - 1)
                if layer == 1:
                    plan_tableD_tile(b - 1)
        # tails
        if layer == 1:
            for t1 in (NB - 2, NB - 1):
                nt1 = min(P, Nc - t1 * P)
                em("sp", (lambda s=t1 % 2, t1=t1, nt1=nt1: (lambda e: e.dma_start(
                    out=hA2s[t1 * P:t1 * P + nt1, :],
                    in_=htile_sb[0:nt1, s, 0, :])))(),
                   dma=True, waits=[("s_dve", m[("D", "ops", t1)])])
            m[("D", "allConf")] = flush("sp")
            plan_cc_chunk(hA2s, hA2f, 0, Nc, m[("D", "allConf")])
        if layer == 2:
            for b1 in (NB - 2, NB - 1):
                nd1 = min(P, Nc - b1 * P)
                em("sp", (lambda sp=b1 % 2, b1=b1, nd1=nd1: (lambda e: e.dma_start(
                    out=out_p[b1 * P:b1 * P + nd1, :],
                    in_=outsb[0:nd1, sp, :])))(),
                   dma=True, waits=[("s_dve", m[(p, "epi", b1)])])
            m[(p, "allConf")] = flush("sp")

    def plan_cc_chunk(hAs, hAf, r0, r1, conf_val):
        em("pool", (lambda hAs=hAs, hAf=hAf, r0=r0, r1=r1: (lambda e: e.collective_compute(
            "AllGather", mybir.AluOpType.bypass,
            replica_groups=[list(range(NCORES))],
            ins=[hAs[r0:r1, :]],
            outs=[hAf[NCORES * r0:NCORES * r1, :]])))(),
           cc=True, waits=[("s_spr", conf_val)])

    # ======================= assemble ========================================
    plan_tableA()
    # L1 table AllGather in 4 chunks, dispatched as quarters of hA1s land
    GROUP_ROWS = GT * P
    if MONO_CC:
        plan_cc_chunk(hA1s, hA1f, 0, Nc, m[("A", "allConf")])
    else:
        for q in range(3):
            r0, r1 = CHUNKS1[q], CHUNKS1[q + 1]
            qg = r1 // GROUP_ROWS
            cv = m[("A", "conf", qg)] if CC_OVERLAP else m[("A", "allConf")]
            plan_cc_chunk(hA1s, hA1f, r0, r1, cv)
        plan_cc_chunk(hA1s, hA1f, CHUNKS1[3], Nc, m[("A", "allConf")])
    barrier()
    plan_edges(1)
    barrier()
    plan_edges(2)
    barrier()

    # ======================= emit ============================================
    lowp = nc.allow_low_precision(reason="bf16 table rows: tolerance 2e-2")
    lowp.__enter__()
    with nc.Block() as block:
        def emit_for(eng_name):
            def runner(eng):
                hwm = {n: 0 for n in sem_names}
                for fn, waits, inc in items[eng_name]:
                    for sname, v in waits:
                        if v > hwm[sname]:
                            eng.wait_ge(sems[sname], v)
                            hwm[sname] = v
                    if fn is not None:
                        inst = fn(eng)
                        if inc is not None:
                            inst.then_inc(sems[inc[0]], inc[1])
            return runner

        block.sync(emit_for("sp"))
        block.gpsimd(emit_for("pool"))
        block.tensor(emit_for("pe"))
        block.vector(emit_for("dve"))
        block.scalar(emit_for("act"))

    lowp.__exit__(None, None, None)
    ctx.close()
    return nc


# ----------------------------------------------------------------------------
# Host-side preparation
# ----------------------------------------------------------------------------

def host_prep(x, edge_index, edge_weight,
              W1, att_src1, att_dst1, W_e1, att_e1, b1,
              W2, att_src2, att_dst2, W_e2, att_e2, b2,
              n_cores=NCORES):
    import ml_dtypes
    BF = ml_dtypes.bfloat16

    N = x.shape[0]
    Nc = N // n_cores
    NB = math.ceil(Nc / 128)

    src0 = np.asarray(edge_index[0], dtype=np.int64)
    dst0 = np.asarray(edge_index[1], dtype=np.int64)
    ew = np.asarray(edge_weight, dtype=np.float32)
    ea_mean = float(ew.mean())
    # self loops are NOT in the stream: chunk 0 of each block handles them
    order = np.argsort(dst0, kind="stable")
    src, dst, ea = src0[order], dst0[order], ew[order]

    core = dst // Nc
    local = dst - core * Nc
    blk = np.minimum(local // 128, NB - 1)
    gid = core * NB + blk
    counts = np.bincount(gid, minlength=n_cores * NB)

    # per-block-index gather chunk count (max over cores), +1 for self chunk
    cpb = counts.reshape(n_cores, NB)
    KCs = [1 + int(np.ceil(cpb[:, b].max() / 128.0)) for b in range(NB)]
    KC = max(max(KCs), 2)

    gstart = np.zeros(n_cores * NB + 1, dtype=np.int64)
    np.cumsum(counts, out=gstart[1:])
    pos = np.arange(src.shape[0], dtype=np.int64) - gstart[gid]

    mdst = np.full((n_cores, NB, 128, KC), SENTINEL, dtype=np.float32)
    mea = np.zeros((n_cores, NB, KC, 128), dtype=np.float32)
    mea[:, :, 0, :] = ea_mean   # self-loop edge attr

    pp = (pos % 128).astype(np.int64)
    cc = 1 + (pos // 128).astype(np.int64)   # gather chunks start at 1
    mdst[core, blk, pp, cc] = (local - blk * 128).astype(np.float32)
    mea[core, blk, cc, pp] = ea

    # gathered-table row index under chunk-major AllGather layout:
    # row(g) = 8*B[q] + srccore*(B[q+1]-B[q]) + (l - B[q]),  l = g % Nc in
    # chunk q of boundaries B.
    def chunked_rows(g, bounds):
        B = np.asarray(list(bounds) + [Nc], dtype=np.int64)
        sc = g // Nc
        l = g % Nc
        q = np.searchsorted(B, l, side="right") - 1
        return (n_cores * B[q] + sc * (B[q + 1] - B[q]) + (l - B[q])).astype(
            np.int32)

    b1_ = (0,) if MONO_CC else CHUNKS1
    b2_ = (0,)
    midx1 = np.zeros((n_cores, NB, 128, KC), dtype=np.int32)
    midx2 = np.zeros((n_cores, NB, 128, KC), dtype=np.int32)
    midx1[core, blk, pp, cc] = chunked_rows(src, b1_)
    midx2[core, blk, pp, cc] = chunked_rows(src, b2_)

    W1 = np.asarray(W1, np.float32)
    W2 = np.asarray(W2, np.float32)
    kr1 = (np.asarray(W_e1, np.float32).reshape(HEADS, OUT_DIM)
           * np.asarray(att_e1, np.float32)).sum(1)
    kr2 = (np.asarray(W_e2, np.float32).reshape(HEADS, OUT_DIM)
           * np.asarray(att_e2, np.float32)).sum(1)
    kap1 = np.zeros((KC, KC * HEADS), np.float32)
    kap2 = np.zeros((KC, KC * HEADS), np.float32)
    for c in range(KC):
        kap1[c, c * HEADS:(c + 1) * HEADS] = kr1
        kap2[c, c * HEADS:(c + 1) * HEADS] = kr2

    rep = lambda v: np.ascontiguousarray(
        np.tile(np.asarray(v, np.float32).reshape(1, HC), (128, 1)))
    iota = np.tile(np.arange(128, dtype=np.float32), (128, 1))
    consts = {
        "W1": np.ascontiguousarray(W1),
        "W2": np.ascontiguousarray(W2),
        "asrc1": rep(att_src1), "adst1": rep(att_dst1),
        "asrc2": rep(att_src2), "adst2": rep(att_dst2),
        "bb1": rep(b1), "bb2": rep(b2),
        "iob": np.ascontiguousarray(iota.astype(BF)),
        "idn": np.ascontiguousarray(np.eye(128, dtype=np.float32)),
        "idb": np.ascontiguousarray(np.eye(128).astype(BF)),
        "kap1": kap1, "kap2": kap2,
    }

    x = np.asarray(x, np.float32)
    in_maps = []
    for c in range(n_cores):
        xs = np.ascontiguousarray(x[c * Nc:(c + 1) * Nc].T)
        in_maps.append({
            "xT": xs,
            "midx1": np.ascontiguousarray(midx1[c]),
            "midx2": np.ascontiguousarray(midx2[c]),
            "mdst": np.ascontiguousarray(mdst[c]),
            "mea": np.ascontiguousarray(mea[c]),
            **consts,
        })
    return in_maps, Nc, KCs


# ----------------------------------------------------------------------------
# public entry
# ----------------------------------------------------------------------------

_RUNNER_CACHE = {}


def _make_runner(nc, n_cores):
    """Reusable jitted shard_map executor for a Bass module (mirrors
    bass2jax.run_bass_via_pjrt but callable repeatedly)."""
    import jax
    from jax.experimental.shard_map import shard_map
    from jax.sharding import Mesh, PartitionSpec
    from concourse import bass2jax

    bass2jax.install_neuronx_cc_hook()
    partition_name = nc.partition_id_tensor.name if nc.partition_id_tensor else None

    in_names, out_names, out_avals, zshapes = [], [], [], []
    for alloc in nc.m.functions[0].allocations:
        if not isinstance(alloc, mybir.MemoryLocationSet):
            continue
        name = alloc.memorylocations[0].name
        if alloc.kind == "ExternalInput":
            if name != partition_name:
                in_names.append(name)
        elif alloc.kind == "ExternalOutput":
            shape = tuple(alloc.tensor_shape)
            dtype = mybir.dt.np(alloc.dtype)
            out_names.append(name)
            out_avals.append(jax.core.ShapedArray(shape, dtype))
            zshapes.append((shape, dtype))

    n_params, n_outs = len(in_names), len(out_names)
    all_in = list(in_names) + list(out_names)
    if partition_name is not None:
        all_in.append(partition_name)
    donate = tuple(range(n_params, n_params + n_outs))

    def _body(*args):
        operands = list(args)
        if partition_name is not None:
            operands.append(bass2jax.partition_id_tensor())
        return tuple(bass2jax._bass_exec_p.bind(
            *operands, out_avals=tuple(out_avals), in_names=tuple(all_in),
            out_names=tuple(out_names), lowering_input_output_aliases=(),
            sim_require_finite=True, sim_require_nnan=True, nc=nc))

    devices = jax.devices()[:n_cores]
    mesh = Mesh(np.asarray(devices), ("core",))
    sharded = jax.jit(
        shard_map(_body, mesh=mesh,
                  in_specs=(PartitionSpec("core"),) * (n_params + n_outs),
                  out_specs=(PartitionSpec("core"),) * n_outs,
                  check_rep=False),
        donate_argnums=donate, keep_unused=True)

    def run(in_maps):
        import jax
        concat_in = [
            np.concatenate([np.asarray(in_maps[c][n]) for c in range(n_cores)],
                           axis=0)
            for n in in_names
        ]
        zeros = [np.zeros((n_cores * s[0], *s[1:]), dt) for (s, dt) in zshapes]
        arrs = sharded(*concat_in, *zeros)
        jax.block_until_ready(arrs)
        return [
            {n: np.asarray(arrs[i]).reshape(n_cores, *out_avals[i].shape)[c]
             for i, n in enumerate(out_names)}
            for c in range(n_cores)
        ]

    return run


def prep_and_runner(**inputs):
    in_maps, Nc, KCs = host_prep(
        inputs["x"], inputs["edge_index"], inputs["edge_weight"],
        inputs["W1"], inputs["att_src1"], inputs["att_dst1"],
        inputs["W_e1"], inputs["att_e1"], inputs["b1"],
        inputs["W2"], inputs["att_src2"], inputs["att_dst2"],
        inputs["W_e2"], inputs["att_e2"], inputs["b2"],
    )
    key = (Nc, tuple(KCs))
    if key not in _RUNNER_CACHE:
        nc = build_gat(Nc, KCs)
        _RUNNER_CACHE[key] = _make_runner(nc, NCORES)
    return _RUNNER_CACHE[key], in_maps


def kernel(**inputs):
    run, in_maps = prep_and_runner(**inputs)
    res = run(in_maps)
    out = np.concatenate([res[c]["out"] for c in range(NCORES)], axis=0)
    return out.astype(np.float32)



# revision 3
# speedup vs baseline: 1.1709x; 1.1709x over previous
"""2-layer GAT (PyG GATConv semantics) on 8 Trainium2 NeuronCores.

Strategy (graph/data parallel, per sharding hint):
  - Nodes partitioned into 8 contiguous ranges (6250 per core).
  - Each core computes h = x @ W plus per-node attention scalars (a_src, a_dst)
    for its own slice, packs them into table rows [h(128) | a_s(4) | a_d(4)],
    and an AllGather replicates the full 50000x136 table to every core.
  - Edges (with self loops) are sorted by destination and assigned to the core
    owning the destination; within a core they are grouped into 49 blocks of
    128 destination slots, each padded to a uniform number of 128-edge chunks.
  - Per chunk, source rows are fetched with an indirect DMA gather (one row per
    partition); the per-destination softmax + weighted aggregation is done with
    one-hot matmuls into PSUM (segment softmax without max subtraction -- exact
    up to fp rounding since alphas are O(10)).
  - Layer 2 repeats the same edge program against the layer-2 table.

Raw Bass (no Tile -- the Tile drain is incompatible with this toolchain),
hand-scheduled with per-engine semaphore counts, 3-deep block pipelining.
One-hot matrices and messages are bf16 (values exact / tolerance-safe).
"""
import sys

sys.path.insert(0, "/opt/trn_rl_repo")

import math
from contextlib import ExitStack

import numpy as np

import concourse.bass as bass
import concourse.mybir as mybir

F32 = mybir.dt.float32
BF16 = mybir.dt.bfloat16
I32 = mybir.dt.int32
P = 128

# problem constants (hardcoded per contract)
N_NODES = 50000
N_EDGES = 800000
IN_DIM = 256
OUT_DIM = 32
HEADS = 4
HC = OUT_DIM * HEADS  # 128
NEG_SLOPE = 0.2
NCORES = 8
TW = HC + 2 * HEADS  # table row width: h | a_src | a_dst = 136
SENTINEL = 320.0     # pad-edge dst_local (bf16-exact, >= 128 so one-hot is 0)
GS = 4               # gather/message/meta pipeline depth
CHUNKS1 = (0, 2048, 4096, 6144)   # L1 table AllGather chunk row starts
CHUNKS2 = (0, 2048, 4096, 5888)   # L2 table AllGather chunk row starts
CC_OVERLAP = True
MONO_CC = False
ABLATE_GATHER = False


# ----------------------------------------------------------------------------
# Bass program builder
# ----------------------------------------------------------------------------

def build_gat(Nc, KCs, in_dim=IN_DIM):
    NB = math.ceil(Nc / 128)  # destination blocks == node tiles
    NT = NB
    KS = in_dim // P
    assert in_dim % P == 0
    assert len(KCs) == NB
    KC = max(KCs)  # array-sizing chunk count (incl. self chunk 0)

    nc = bass.Bass()

    # ---- DRAM parameters ----
    xT = nc.declare_dram_parameter("xT", [in_dim, Nc], F32, isOutput=False)
    W1 = nc.declare_dram_parameter("W1", [in_dim, HC], F32, isOutput=False)
    W2 = nc.declare_dram_parameter("W2", [HC, HC], F32, isOutput=False)
    asrc1 = nc.declare_dram_parameter("asrc1", [P, HC], F32, isOutput=False)
    adst1 = nc.declare_dram_parameter("adst1", [P, HC], F32, isOutput=False)
    asrc2 = nc.declare_dram_parameter("asrc2", [P, HC], F32, isOutput=False)
    adst2 = nc.declare_dram_parameter("adst2", [P, HC], F32, isOutput=False)
    bb1 = nc.declare_dram_parameter("bb1", [P, HC], F32, isOutput=False)
    bb2 = nc.declare_dram_parameter("bb2", [P, HC], F32, isOutput=False)
    iob = nc.declare_dram_parameter("iob", [P, P], BF16, isOutput=False)
    idn = nc.declare_dram_parameter("idn", [P, P], F32, isOutput=False)
    idb = nc.declare_dram_parameter("idb", [P, P], BF16, isOutput=False)
    kap1 = nc.declare_dram_parameter("kap1", [KC, KC * HEADS], F32, isOutput=False)
    kap2 = nc.declare_dram_parameter("kap2", [KC, KC * HEADS], F32, isOutput=False)
    midx1 = nc.declare_dram_parameter("midx1", [NB, P, KC], I32, isOutput=False)
    midx2 = nc.declare_dram_parameter("midx2", [NB, P, KC], I32, isOutput=False)
    mdst = nc.declare_dram_parameter("mdst", [NB, P, KC], F32, isOutput=False)
    mea = nc.declare_dram_parameter("mea", [NB, KC, P], F32, isOutput=False)
    out_p = nc.declare_dram_parameter("out", [Nc, HC], F32, isOutput=True)

    # ---- internal DRAM ----
    hA1s = nc.dram_tensor("hA1s", [Nc, TW], F32)
    hA2s = nc.dram_tensor("hA2s", [Nc, TW], F32)
    hA1f = nc.dram_tensor("hA1f", [Nc * NCORES, TW], F32, addr_space="Shared")
    hA2f = nc.dram_tensor("hA2f", [Nc * NCORES, TW], F32, addr_space="Shared")

    ctx = ExitStack()
    sb = lambda name, shape, dt=F32: ctx.enter_context(
        nc.sbuf_tensor(name, shape, dt))
    ps = lambda name, shape, dt=F32: ctx.enter_context(
        nc.psum_tensor(name, shape, dt))

    # ---- SBUF ----
    W1_sb = sb("W1_sb", [P, KS, P])
    W2_sb = sb("W2_sb", [P, HC])
    asrc1_sb = sb("asrc1_sb", [P, HC]); adst1_sb = sb("adst1_sb", [P, HC])
    asrc2_sb = sb("asrc2_sb", [P, HC]); adst2_sb = sb("adst2_sb", [P, HC])
    bb1_sb = sb("bb1_sb", [P, HC]); bb2_sb = sb("bb2_sb", [P, HC])
    iob_sb = sb("iob_sb", [P, P], BF16)
    idn_sb = sb("idn_sb", [P, P]); idb_sb = sb("idb_sb", [P, P], BF16)
    kap1_sb = sb("kap1_sb", [KC, KC * HEADS])
    kap2_sb = sb("kap2_sb", [KC, KC * HEADS])
    xk_sb = sb("xk_sb", [P, 2, KS, 4 * P])
    htile_sb = sb("htile_sb", [P, 2, 4, TW])
    tmp_sb = sb("tmp_sb", [P, 4, HC])
    h1_sb = sb("h1_sb", [P, NT * P])
    h1T_sb = sb("h1T_sb", [P, 2, P])
    g_sb = sb("g_sb", [P, GS, KC, TW])
    msg_sb = sb("msg_sb", [P, GS, KC, TW], BF16)
    M_sb = sb("M_sb", [P, 2, KC, P], BF16)
    Mt_sb = sb("Mt_sb", [P, 2, KC, P], BF16)
    idx_sb = sb("idx_sb", [P, GS, KC], I32)
    dst_sb = sb("dst_sb", [P, GS, KC])
    eaT_sb = sb("eaT_sb", [KC, GS, P])
    adbb_sb = sb("adbb_sb", [P, GS, HEADS], BF16)
    alpha_sb = sb("alpha_sb", [P, 2, KC, HEADS])
    alph2_sb = sb("alph2_sb", [P, KC, HEADS])
    asg_sb = sb("asg_sb", [P, 2, KC, HEADS])
    recip_sb = sb("recip_sb", [P, HEADS])
    outsb = sb("outsb", [P, 2, HC])

    # ---- PSUM (8 banks) ----
    ph = ps("ph0", [P, 4 * P])                    # h matmuls (A, D), 1 bank
    ptf = ps("ptf", [P, P])                       # f32 transposes (phase D)
    ptb = [ps("ptb0", [P, P], BF16), ps("ptb1", [P, P], BF16)]  # M transposes
    pa = [ps("pa0", [P, KC * HEADS]), ps("pa1", [P, KC * HEADS])]
    po = [ps("po0", [P, HC + HEADS]), ps("po1", [P, HC + HEADS])]

    # ---- semaphores / plan ----
    sem_names = ["s_sp", "s_pool", "s_pe", "s_dve", "s_act", "s_cc",
                 "s_spr", "s_plr", "s_g0", "s_g1", "s_g2", "s_g3"]
    sems = {n: ctx.enter_context(nc.semaphore(n)) for n in sem_names}
    ENG_SEM = {"sp": "s_sp", "pool": "s_pool", "pe": "s_pe", "dve": "s_dve",
               "act": "s_act"}
    RDY_SEM = {"sp": "s_spr", "pool": "s_plr"}

    cnt = {n: 0 for n in sem_names}
    items = {e: [] for e in ENG_SEM}
    m = {}

    def em(eng, fn=None, dma=False, cc=False, raw=False, waits=(), inc=None):
        if fn is not None and not raw:
            if inc is not None:
                pass
            elif cc:
                inc = ("s_cc", 1)
            elif dma:
                inc = (ENG_SEM[eng], 16)
            else:
                inc = (ENG_SEM[eng], 1)
            cnt[inc[0]] += inc[1]
        else:
            inc = None
        items[eng].append((fn, [(s, v) for (s, v) in waits if v > 0], inc))

    def drain(eng):
        em(eng, lambda e: e.drain(), raw=True)

    def flush(eng):
        rs = RDY_SEM[eng]
        em(eng, None, waits=[(ENG_SEM[eng], cnt[ENG_SEM[eng]])])
        em(eng, (lambda rs=rs: (lambda e: e.sem_inc(sems[rs], 1)))(), raw=True)
        cnt[rs] += 1
        return cnt[rs]

    def barrier():
        snap = dict(cnt)
        for e in ENG_SEM:
            em(e, None, waits=[(s, snap[s]) for s in sem_names])

    AL = mybir.AluOpType
    AF = mybir.ActivationFunctionType
    AX = mybir.AxisListType

    # ======================= const loads =======================
    def cdma(dst_ap, src_ap):
        em("sp", (lambda d=dst_ap, s=src_ap: (lambda e: e.dma_start(out=d, in_=s)))(),
           dma=True)

    for k in range(KS):
        cdma(W1_sb[:, k, :], W1[k * P:(k + 1) * P, :])
    cdma(W2_sb[:], W2[:])
    cdma(asrc1_sb[:], asrc1[:]); cdma(adst1_sb[:], adst1[:])
    cdma(asrc2_sb[:], asrc2[:]); cdma(adst2_sb[:], adst2[:])
    cdma(bb1_sb[:], bb1[:]); cdma(bb2_sb[:], bb2[:])
    cdma(iob_sb[:], iob[:]); cdma(idn_sb[:], idn[:]); cdma(idb_sb[:], idb[:])
    cdma(kap1_sb[:], kap1[:]); cdma(kap2_sb[:], kap2[:])
    m["constsConf"] = flush("sp")
    # self-chunk rows beyond nd of the last block are read (harmlessly) --
    # give them defined values once
    em("dve", lambda e: e.memset(g_sb[:], 0.0))

    # ======================= table phase A (batched groups of 4 tiles) ======
    GT = 4                      # tiles per group
    NG = math.ceil(Nc / (GT * P))

    def plan_tableA():
        p = "A"
        for g in range(NG):
            c0 = g * GT * P
            ng = min(GT * P, Nc - c0)
            jt = math.ceil(ng / P)          # tiles in this group
            s = g % 2
            w = [("s_pe", m.get((p, "mm", g - 2), 0))]
            for k in range(KS):
                em("sp", (lambda s=s, k=k, c0=c0, ng=ng: (lambda e: e.dma_start(
                    out=xk_sb[:, s, k, 0:ng],
                    in_=xT[k * P:(k + 1) * P, c0:c0 + ng])))(),
                   dma=True, waits=w if k == 0 else ())
            m[(p, "conf", g)] = flush("sp")
            if g >= 1:
                g1 = g - 1
                c1 = g1 * GT * P
                ng1 = min(GT * P, Nc - c1)
                jt1 = math.ceil(ng1 / P)
                if ng1 == GT * P:
                    dst_ap = hA1s[c1:c1 + ng1, :].rearrange(
                        "(j q) c -> q j c", q=P)
                    src_ap = htile_sb[:, 1 - s, :, :]
                else:
                    dst_ap = hA1s[c1:c1 + ng1, :]
                    src_ap = htile_sb[0:ng1, 1 - s, 0, :]
                em("sp", (lambda d=dst_ap, sr=src_ap: (lambda e: e.dma_start(
                    out=d, in_=sr)))(),
                   dma=True, waits=[("s_dve", m[(p, "ops", g1)])])
            w = [("s_spr", m[(p, "conf", g)]),
                 ("s_dve", m.get((p, "ops", g - 1), 0))]
            first = True
            for j in range(jt):
                nt = min(P, ng - j * P)
                for k in range(KS):
                    em("pe", (lambda s=s, k=k, j=j, nt=nt, c0=c0: (lambda e: e.matmul(
                        out=ph[0:nt, j * P:j * P + P],
                        lhsT=xk_sb[:, s, k, j * P:j * P + nt],
                        rhs=W1_sb[:, k, :], start=(k == 0), stop=(k == KS - 1),
                        skip_group_check=True)))(),
                       waits=w if first else ())
                    first = False
            m[(p, "mm", g)] = cnt["s_pe"]
            # DVE: pack table rows
            w = [("s_pe", m[(p, "mm", g)]),
                 ("s_spr", m.get((p, "conf", g), 0) if g >= 2 else 0)]
            if ng == GT * P:
                em("dve", (lambda s=s, jt=jt: (lambda e: e.tensor_copy(
                    out=htile_sb[:, s, 0:jt, 0:HC],
                    in_=ph[:, 0:jt * P].rearrange(
                        "q (j c) -> q j c", j=jt))))(), waits=w)
                m[(p, "copy", g)] = cnt["s_dve"]
                drain("dve")
                em("dve", (lambda s=s, jt=jt: (lambda e: e.tensor_mul(
                    out=tmp_sb[:, 0:jt, :],
                    in0=ph[:, 0:jt * P].rearrange("q (j c) -> q j c", j=jt),
                    in1=asrc1_sb[:, None, :].to_broadcast([P, jt, HC]))))())
                drain("dve")
                em("dve", (lambda s=s, jt=jt: (lambda e: e.tensor_reduce(
                    out=htile_sb[:, s, 0:jt, HC:HC + HEADS],
                    in_=tmp_sb[:, 0:jt, :].rearrange(
                        "q j (h c) -> q j h c", c=OUT_DIM),
                    axis=AX.X, op=AL.add)))())
                drain("dve")
                em("dve", (lambda s=s, jt=jt: (lambda e: e.tensor_mul(
                    out=tmp_sb[:, 0:jt, :],
                    in0=ph[:, 0:jt * P].rearrange("q (j c) -> q j c", j=jt),
                    in1=adst1_sb[:, None, :].to_broadcast([P, jt, HC]))))())
                drain("dve")
                em("dve", (lambda s=s, jt=jt: (lambda e: e.tensor_reduce(
                    out=htile_sb[:, s, 0:jt, HC + HEADS:TW],
                    in_=tmp_sb[:, 0:jt, :].rearrange(
                        "q j (h c) -> q j h c", c=OUT_DIM),
                    axis=AX.X, op=AL.add)))())
            else:
                first = True
                for j in range(jt):
                    nt = min(P, ng - j * P)
                    em("dve", (lambda s=s, j=j, nt=nt: (lambda e: e.tensor_copy(
                        out=htile_sb[0:nt, s, j, 0:HC],
                        in_=ph[0:nt, j * P:j * P + P])))(),
                       waits=w if first else ())
                    first = False
                    if j == 0:
                        m[(p, "copy", g)] = cnt["s_dve"]
                    drain("dve")
                    em("dve", (lambda s=s, j=j, nt=nt: (lambda e: e.tensor_mul(
                        out=tmp_sb[0:nt, 0, :],
                        in0=ph[0:nt, j * P:j * P + P],
                        in1=asrc1_sb[0:nt, :])))())
                    drain("dve")
                    em("dve", (lambda s=s, j=j, nt=nt: (lambda e: e.tensor_reduce(
                        out=htile_sb[0:nt, s, j, HC:HC + HEADS],
                        in_=tmp_sb[0:nt, 0, :].rearrange(
                            "q (h c) -> q h c", c=OUT_DIM),
                        axis=AX.X, op=AL.add)))())
                    drain("dve")
                    em("dve", (lambda s=s, j=j, nt=nt: (lambda e: e.tensor_mul(
                        out=tmp_sb[0:nt, 0, :],
                        in0=ph[0:nt, j * P:j * P + P],
                        in1=adst1_sb[0:nt, :])))())
                    drain("dve")
                    em("dve", (lambda s=s, j=j, nt=nt: (lambda e: e.tensor_reduce(
                        out=htile_sb[0:nt, s, j, HC + HEADS:TW],
                        in_=tmp_sb[0:nt, 0, :].rearrange(
                            "q (h c) -> q h c", c=OUT_DIM),
                        axis=AX.X, op=AL.add)))())
            m[(p, "ops", g)] = cnt["s_dve"]
        g = NG - 1
        c0 = g * GT * P
        ng = min(GT * P, Nc - c0)
        if ng == GT * P:
            dst_ap = hA1s[c0:c0 + ng, :].rearrange("(j q) c -> q j c", q=P)
            src_ap = htile_sb[:, g % 2, :, :]
        else:
            jt = math.ceil(ng / P)
            dst_ap = hA1s[c0:c0 + ng, :]
            src_ap = None  # handled below per-tile for ragged tail
        if src_ap is not None:
            em("sp", (lambda d=dst_ap, sr=src_ap: (lambda e: e.dma_start(
                out=d, in_=sr)))(),
               dma=True, waits=[("s_dve", m[(p, "ops", g)])])
        else:
            # ragged: store tile by tile
            w = [("s_dve", m[(p, "ops", g)])]
            for j in range(math.ceil(ng / P)):
                nt = min(P, ng - j * P)
                em("sp", (lambda s=g % 2, j=j, nt=nt, c0=c0: (lambda e: e.dma_start(
                    out=hA1s[c0 + j * P:c0 + j * P + nt, :],
                    in_=htile_sb[0:nt, s, j, :])))(),
                   dma=True, waits=w if j == 0 else ())
        m[(p, "allConf")] = flush("sp")

    # --- layer-2 table tile (merged into layer-1 edge pipeline) ---
    def plan_tableD_tile(t):
        p = "D"
        s = t % 2
        em("pe", (lambda t=t: (lambda e: e.transpose(
            out=ptf[:], in_=h1_sb[:, t * P:(t + 1) * P],
            identity=idn_sb[:])))(),
           waits=[("s_dve", m[("C", "epi", t)]),
                  ("s_dve", m.get((p, "c1", t - 1), 0))])
        m[(p, "T", t)] = cnt["s_pe"]
        em("dve", (lambda s=s: (lambda e: e.tensor_copy(
            out=h1T_sb[:, s, :], in_=ptf[:])))(),
           waits=[("s_pe", m[(p, "T", t)]),
                  ("s_pe", m.get((p, "mm", t - 2), 0))])
        m[(p, "c1", t)] = cnt["s_dve"]
        em("pe", (lambda s=s: (lambda e: e.matmul(
            out=ph[:, 0:HC], lhsT=h1T_sb[:, s, :], rhs=W2_sb[:],
            start=True, stop=True)))(),
           waits=[("s_dve", m[(p, "c1", t)]),
                  ("s_dve", m.get((p, "ops", t - 1), 0))])
        m[(p, "mm", t)] = cnt["s_pe"]
        nt = min(P, Nc - t * P)
        w = [("s_pe", m[(p, "mm", t)]),
             ("s_spr", m.get((p, "stconf", t - 2), 0))]
        em("dve", (lambda s=s: (lambda e: e.tensor_copy(
            out=htile_sb[:, s, 0, 0:HC], in_=ph[:, 0:HC])))(), waits=w)
        m[(p, "copy", t)] = cnt["s_dve"]
        drain("dve")
        em("dve", (lambda s=s: (lambda e: e.tensor_mul(
            out=tmp_sb[:, 0, :], in0=ph[:, 0:HC],
            in1=asrc2_sb[:])))())
        drain("dve")
        em("dve", (lambda s=s: (lambda e: e.tensor_reduce(
            out=htile_sb[:, s, 0, HC:HC + HEADS],
            in_=tmp_sb[:, 0, :].rearrange("q (h c) -> q h c", c=OUT_DIM),
            axis=AX.X, op=AL.add)))())
        drain("dve")
        em("dve", (lambda s=s: (lambda e: e.tensor_mul(
            out=tmp_sb[:, 0, :], in0=ph[:, 0:HC],
            in1=adst2_sb[:])))())
        drain("dve")
        em("dve", (lambda s=s: (lambda e: e.tensor_reduce(
            out=htile_sb[:, s, 0, HC + HEADS:TW],
            in_=tmp_sb[:, 0, :].rearrange("q (h c) -> q h c", c=OUT_DIM),
            axis=AX.X, op=AL.add)))())
        m[(p, "ops", t)] = cnt["s_dve"]

    # ======================= edge phase (C: layer1, F: layer2) ===============
    # Software-pipelined: block b's gathers are issued in loop iter b, its
    # compute consumption happens in iter b+1, and the Pool confirmation flush
    # runs only every second block so the DMA ring never fully drains per
    # block. Chunk 0 of each block is the self-loop chunk (regular DMA,
    # identity one-hot, doubles as a_dst source + denominator guarantee).
    def plan_edges(layer):
        p = "C" if layer == 1 else "F"
        hAf = hA1f if layer == 1 else hA2f
        hAs = hA1s if layer == 1 else hA2s
        midx = midx1 if layer == 1 else midx2
        kap_sb = kap1_sb if layer == 1 else kap2_sb
        bias_sb = bb1_sb if layer == 1 else bb2_sb

        def plan_meta(b):
            nd = min(P, Nc - b * P)
            s = b % GS
            w = [(f"s_g{b % GS}", m.get((p, "gcnt", b - GS), 0)),
                 ("s_pe", m.get((p, "scat", b - GS), 0))]
            em("sp", (lambda s=s, b=b: (lambda e: e.dma_start(
                out=idx_sb[:, s, :], in_=midx[b])))(), dma=True, waits=w)
            em("sp", (lambda s=s, b=b: (lambda e: e.dma_start(
                out=dst_sb[:, s, :], in_=mdst[b])))(), dma=True)
            em("sp", (lambda s=s, b=b: (lambda e: e.dma_start(
                out=eaT_sb[:, s, :], in_=mea[b])))(), dma=True)
            em("sp", (lambda s=s, b=b, nd=nd, hAs=hAs: (lambda e: e.dma_start(
                out=g_sb[0:nd, s, 0, :],
                in_=hAs[b * P:b * P + nd, :])))(), dma=True)
            if layer == 1 and b >= 2:
                t1 = b - 2
                nt1 = min(P, Nc - t1 * P)
                em("sp", (lambda s=t1 % 2, t1=t1, nt1=nt1: (lambda e: e.dma_start(
                    out=hA2s[t1 * P:t1 * P + nt1, :],
                    in_=htile_sb[0:nt1, s, 0, :])))(),
                   dma=True, waits=[("s_dve", m[("D", "ops", t1)])])
            if layer == 2 and b >= 2:
                b1 = b - 2
                nd1 = min(P, Nc - b1 * P)
                em("sp", (lambda sp=b1 % 2, b1=b1, nd1=nd1: (lambda e: e.dma_start(
                    out=out_p[b1 * P:b1 * P + nd1, :],
                    in_=outsb[0:nd1, sp, :])))(),
                   dma=True, waits=[("s_dve", m[(p, "epi", b1)])])
            m[(p, "conf", b)] = flush("sp")
            if layer == 1 and b >= 2:
                m[("D", "stconf", b - 2)] = m[(p, "conf", b)]

        def plan_gather(b):
            KCb = KCs[b]
            s = b % GS
            sg = f"s_g{s}"
            w = [("s_spr", m[(p, "conf", b)]),
                 ("s_dve", m.get((p, "msgs", b - GS), 0))]
            for c in range(1, KCb):
                if ABLATE_GATHER:
                    break
                em("pool", (lambda s=s, c=c, hAf=hAf: (lambda e: e.indirect_dma_start(
                    out=g_sb[:, s, c, :], out_offset=None, in_=hAf[:],
                    in_offset=bass.IndirectOffsetOnAxis(
                        ap=idx_sb[:, s, c:c + 1], axis=0))))(),
                   dma=True, waits=w if c == 1 else (), inc=(sg, 16))
            m[(p, "gcnt", b)] = cnt[sg]

        def plan_consume(b):
            KCb = KCs[b]
            s = b % GS
            s2 = b % 2
            # DVE: adb cast + one-hot builds
            w = [("s_spr", m[(p, "conf", b)]),
                 ("s_pe", m.get((p, "scat", b - 2), 0))]
            em("dve", (lambda s=s: (lambda e: e.tensor_copy(
                out=adbb_sb[:, s, :], in_=g_sb[:, s, 0, HC + HEADS:TW])))(),
               waits=w)
            for c in range(1, KCb):
                em("dve", (lambda s=s, s2=s2, c=c: (lambda e: e.tensor_scalar(
                    out=M_sb[:, s2, c, :], in0=iob_sb[:],
                    scalar1=dst_sb[:, s, c:c + 1], scalar2=None,
                    op0=AL.is_equal)))())
                m[(p, "Mb", b, c)] = cnt["s_dve"]
            for c in range(1, KCb):
                em("pe", (lambda s2=s2, c=c: (lambda e: e.transpose(
                    out=ptb[c % 2][:], in_=M_sb[:, s2, c, :],
                    identity=idb_sb[:])))(),
                   waits=[("s_dve", m[(p, "Mb", b, c)]),
                          ("s_dve", m.get((p, "Mtc", b, c - 2),
                                          m.get((p, "MtcPrev", b), 0)))])
                m[(p, "T", b, c)] = cnt["s_pe"]
                em("dve", (lambda s2=s2, c=c: (lambda e: e.tensor_copy(
                    out=Mt_sb[:, s2, c, :], in_=ptb[c % 2][:])))(),
                   waits=[("s_pe", m[(p, "T", b, c)])])
                m[(p, "Mtc", b, c)] = cnt["s_dve"]
            m[(p, "MtcPrev", b + 1)] = m[(p, "Mtc", b, KCb - 1)]
            # PE: alpha accumulation
            em("pe", (lambda s=s, s2=s2, KCb=KCb, kap_sb=kap_sb: (lambda e: e.matmul(
                out=pa[s2][:, 0:KCb * HEADS],
                lhsT=eaT_sb[0:KCb, s, :], rhs=kap_sb[0:KCb, 0:KCb * HEADS],
                start=True, stop=False, skip_group_check=True)))(),
               waits=[("s_spr", m[(p, "conf", b)]),
                      ("s_dve", m.get((p, "alpha", b - 2), 0))])
            em("pe", (lambda s=s, s2=s2: (lambda e: e.matmul(
                out=pa[s2][:, 0:HEADS], lhsT=idb_sb[:], rhs=adbb_sb[:, s, :],
                start=False, stop=False, skip_group_check=True)))(),
               waits=[("s_dve", m[(p, "MtcPrev", b + 1)])])
            for c in range(1, KCb):
                em("pe", (lambda s=s, s2=s2, c=c, KCb=KCb: (lambda e: e.matmul(
                    out=pa[s2][:, c * HEADS:(c + 1) * HEADS],
                    lhsT=Mt_sb[:, s2, c, :], rhs=adbb_sb[:, s, :],
                    start=False, stop=(c == KCb - 1), skip_group_check=True)))())
            m[(p, "admm", b)] = cnt["s_pe"]
            # DVE: alpha + leaky relu
            em("dve", (lambda s=s, s2=s2, KCb=KCb: (lambda e: e.tensor_add(
                out=alpha_sb[:, s2, 0:KCb, :],
                in0=g_sb[:, s, 0:KCb, HC:HC + HEADS],
                in1=pa[s2][:, 0:KCb * HEADS].rearrange(
                    "p (k h) -> p k h", h=HEADS))))(),
               waits=[("s_pe", m[(p, "admm", b)]),
                      (f"s_g{b % GS}", m[(p, "gcnt", b)])])
            drain("dve")
            em("dve", (lambda s2=s2, KCb=KCb: (lambda e: e.tensor_scalar(
                out=alph2_sb[:, 0:KCb, :], in0=alpha_sb[:, s2, 0:KCb, :],
                scalar1=NEG_SLOPE, scalar2=None, op0=AL.mult)))())
            drain("dve")
            em("dve", (lambda s2=s2, KCb=KCb: (lambda e: e.tensor_tensor(
                out=alpha_sb[:, s2, 0:KCb, :], in0=alpha_sb[:, s2, 0:KCb, :],
                in1=alph2_sb[:, 0:KCb, :], op=AL.max)))())
            m[(p, "alpha", b)] = cnt["s_dve"]
            # ACT: exp
            em("act", (lambda s=s, s2=s2, KCb=KCb: (lambda e: e.activation(
                out=msg_sb[:, s, 0:KCb, HC:HC + HEADS],
                in_=alpha_sb[:, s2, 0:KCb, :], func=AF.Exp)))(),
               waits=[("s_dve", m[(p, "alpha", b)]),
                      ("s_pe", m.get((p, "scat", b - GS), 0))])
            m[(p, "exp", b)] = cnt["s_act"]
            # DVE: messages
            em("dve", (lambda s=s, KCb=KCb: (lambda e: e.tensor_mul(
                out=msg_sb[:, s, 0:KCb, 0:HC].rearrange(
                    "p k (h c) -> p k h c", c=OUT_DIM),
                in0=g_sb[:, s, 0:KCb, 0:HC].rearrange(
                    "p k (h c) -> p k h c", c=OUT_DIM),
                in1=msg_sb[:, s, 0:KCb, HC:HC + HEADS][:, :, :, None].to_broadcast(
                    [P, KCb, HEADS, OUT_DIM]))))(),
               waits=[("s_act", m[(p, "exp", b)])])
            m[(p, "msgs", b)] = cnt["s_dve"]
            # PE: scatter
            w = [("s_dve", m[(p, "msgs", b)]),
                 ("s_dve", m.get((p, "epi", b - 2), 0))]
            em("pe", (lambda s=s, s2=s2: (lambda e: e.matmul(
                out=po[s2][:], lhsT=idb_sb[:],
                rhs=msg_sb[:, s, 0, 0:HC + HEADS],
                start=True, stop=False)))(), waits=w)
            for c in range(1, KCb):
                em("pe", (lambda s=s, s2=s2, c=c, KCb=KCb: (lambda e: e.matmul(
                    out=po[s2][:], lhsT=M_sb[:, s2, c, :],
                    rhs=msg_sb[:, s, c, 0:HC + HEADS],
                    start=False, stop=(c == KCb - 1))))())
            m[(p, "scat", b)] = cnt["s_pe"]
            # DVE: epilogue
            w = [("s_pe", m[(p, "scat", b)])]
            if layer == 2:
                w.append(("s_spr", m[(p, "conf", b)]))
            em("dve", (lambda s2=s2: (lambda e: e.reciprocal(
                out=recip_sb[:], in_=po[s2][:, HC:HC + HEADS])))(), waits=w)
            if layer == 1:
                tgt = lambda b=b: h1_sb[:, b * P:(b + 1) * P]
            else:
                tgt = lambda s2=s2: outsb[:, s2, :]
            drain("dve")
            em("dve", (lambda s2=s2, tgt=tgt: (lambda e: e.tensor_mul(
                out=tgt().rearrange("p (h c) -> p h c", c=OUT_DIM),
                in0=po[s2][:, 0:HC].rearrange("p (h c) -> p h c", c=OUT_DIM),
                in1=recip_sb[:][:, :, None].to_broadcast(
                    [P, HEADS, OUT_DIM]))))())
            drain("dve")
            em("dve", (lambda tgt=tgt, bias_sb=bias_sb: (lambda e: e.tensor_add(
                out=tgt(), in0=tgt(), in1=bias_sb[:])))())
            m[(p, "epi", b)] = cnt["s_dve"]

        l2cc = {}
        for b in range(NB + 1):
            if b < NB:
                plan_meta(b)
                plan_gather(b)
                if b in l2cc:
                    r0, r1 = l2cc[b]
                    plan_cc_chunk(hA2s, hA2f, r0, r1, m[(p, "conf", b)])
            if b >= 1:
                plan_consume(b - 1)
                if layer == 1:
                    plan_tableD_tile(b - 1)
        # tails
        if layer == 1:
            for t1 in (NB - 2, NB - 1):
                nt1 = min(P, Nc - t1 * P)
                em("sp", (lambda s=t1 % 2, t1=t1, nt1=nt1: (lambda e: e.dma_start(
                    out=hA2s[t1 * P:t1 * P + nt1, :],
                    in_=htile_sb[0:nt1, s, 0, :])))(),
                   dma=True, waits=[("s_dve", m[("D", "ops", t1)])])
            m[("D", "allConf")] = flush("sp")
            plan_cc_chunk(hA2s, hA2f, 0, Nc, m[("D", "allConf")])
        if layer == 2:
            for b1 in (NB - 2, NB - 1):
                nd1 = min(P, Nc - b1 * P)
                em("sp", (lambda sp=b1 % 2, b1=b1, nd1=nd1: (lambda e: e.dma_start(
                    out=out_p[b1 * P:b1 * P + nd1, :],
                    in_=outsb[0:nd1, sp, :])))(),
                   dma=True, waits=[("s_dve", m[(p, "epi", b1)])])
            m[(p, "allConf")] = flush("sp")

    def plan_cc_chunk(hAs, hAf, r0, r1, conf_val):
        em("pool", (lambda hAs=hAs, hAf=hAf, r0=r0, r1=r1: (lambda e: e.collective_compute(
            "AllGather", mybir.AluOpType.bypass,
            replica_groups=[list(range(NCORES))],
            ins=[hAs[r0:r1, :]],
            outs=[hAf[NCORES * r0:NCORES * r1, :]])))(),
           cc=True, waits=[("s_spr", conf_val)])

    # ======================= assemble ========================================
    plan_tableA()
    # L1 table AllGather in 4 chunks, dispatched as quarters of hA1s land
    GROUP_ROWS = GT * P
    if MONO_CC:
        plan_cc_chunk(hA1s, hA1f, 0, Nc, m[("A", "allConf")])
    else:
        for q in range(3):
            r0, r1 = CHUNKS1[q], CHUNKS1[q + 1]
            qg = r1 // GROUP_ROWS
            cv = m[("A", "conf", qg)] if CC_OVERLAP else m[("A", "allConf")]
            plan_cc_chunk(hA1s, hA1f, r0, r1, cv)
        plan_cc_chunk(hA1s, hA1f, CHUNKS1[3], Nc, m[("A", "allConf")])
    barrier()
    plan_edges(1)
    barrier()
    plan_edges(2)
    barrier()

    # ======================= emit ============================================
    lowp = nc.allow_low_precision(reason="bf16 table rows: tolerance 2e-2")
    lowp.__enter__()
    with nc.Block() as block:
        def emit_for(eng_name):
            def runner(eng):
                hwm = {n: 0 for n in sem_names}
                for fn, waits, inc in items[eng_name]:
                    for sname, v in waits:
                        if v > hwm[sname]:
                            eng.wait_ge(sems[sname], v)
                            hwm[sname] = v
                    if fn is not None:
                        inst = fn(eng)
                        if inc is not None:
                            inst.then_inc(sems[inc[0]], inc[1])
            return runner

        block.sync(emit_for("sp"))
        block.gpsimd(emit_for("pool"))
        block.tensor(emit_for("pe"))
        block.vector(emit_for("dve"))
        block.scalar(emit_for("act"))

    lowp.__exit__(None, None, None)
    ctx.close()
    return nc


# ----------------------------------------------------------------------------
# Host-side preparation
# ----------------------------------------------------------------------------

def host_prep(x, edge_index, edge_weight,
              W1, att_src1, att_dst1, W_e1, att_e1, b1,
              W2, att_src2, att_dst2, W_e2, att_e2, b2,
              n_cores=NCORES):
    import ml_dtypes
    BF = ml_dtypes.bfloat16

    N = x.shape[0]
    Nc = N // n_cores
    NB = math.ceil(Nc / 128)

    src0 = np.asarray(edge_index[0], dtype=np.int64)
    dst0 = np.asarray(edge_index[1], dtype=np.int64)
    ew = np.asarray(edge_weight, dtype=np.float32)
    ea_mean = float(ew.mean())
    # self loops are NOT in the stream: chunk 0 of each block handles them
    order = np.argsort(dst0, kind="stable")
    src, dst, ea = src0[order], dst0[order], ew[order]

    core = dst // Nc
    local = dst - core * Nc
    blk = np.minimum(local // 128, NB - 1)
    gid = core * NB + blk
    counts = np.bincount(gid, minlength=n_cores * NB)

    # per-block-index gather chunk count (max over cores), +1 for self chunk
    cpb = counts.reshape(n_cores, NB)
    KCs = [1 + int(np.ceil(cpb[:, b].max() / 128.0)) for b in range(NB)]
    KC = max(max(KCs), 2)

    gstart = np.zeros(n_cores * NB + 1, dtype=np.int64)
    np.cumsum(counts, out=gstart[1:])
    pos = np.arange(src.shape[0], dtype=np.int64) - gstart[gid]

    mdst = np.full((n_cores, NB, 128, KC), SENTINEL, dtype=np.float32)
    mea = np.zeros((n_cores, NB, KC, 128), dtype=np.float32)
    mea[:, :, 0, :] = ea_mean   # self-loop edge attr

    pp = (pos % 128).astype(np.int64)
    cc = 1 + (pos // 128).astype(np.int64)   # gather chunks start at 1
    mdst[core, blk, pp, cc] = (local - blk * 128).astype(np.float32)
    mea[core, blk, cc, pp] = ea

    # gathered-table row index under chunk-major AllGather layout:
    # row(g) = 8*B[q] + srccore*(B[q+1]-B[q]) + (l - B[q]),  l = g % Nc in
    # chunk q of boundaries B.
    def chunked_rows(g, bounds):
        B = np.asarray(list(bounds) + [Nc], dtype=np.int64)
        sc = g // Nc
        l = g % Nc
        q = np.searchsorted(B, l, side="right") - 1
        return (n_cores * B[q] + sc * (B[q + 1] - B[q]) + (l - B[q])).astype(
            np.int32)

    b1_ = (0,) if MONO_CC else CHUNKS1
    b2_ = (0,)
    midx1 = np.zeros((n_cores, NB, 128, KC), dtype=np.int32)
    midx2 = np.zeros((n_cores, NB, 128, KC), dtype=np.int32)
    midx1[core, blk, pp, cc] = chunked_rows(src, b1_)
    midx2[core, blk, pp, cc] = chunked_rows(src, b2_)

    W1 = np.asarray(W1, np.float32)
    W2 = np.asarray(W2, np.float32)
    kr1 = (np.asarray(W_e1, np.float32).reshape(HEADS, OUT_DIM)
           * np.asarray(att_e1, np.float32)).sum(1)
    kr2 = (np.asarray(W_e2, np.float32).reshape(HEADS, OUT_DIM)
           * np.asarray(att_e2, np.float32)).sum(1)
    kap1 = np.zeros((KC, KC * HEADS), np.float32)
    kap2 = np.zeros((KC, KC * HEADS), np.float32)
    for c in range(KC):
        kap1[c, c * HEADS:(c + 1) * HEADS] = kr1
        kap2[c, c * HEADS:(c + 1) * HEADS] = kr2

    rep = lambda v: np.ascontiguousarray(
        np.tile(np.asarray(v, np.float32).reshape(1, HC), (128, 1)))
    iota = np.tile(np.arange(128, dtype=np.float32), (128, 1))
    consts = {
        "W1": np.ascontiguousarray(W1),
        "W2": np.ascontiguousarray(W2),
        "asrc1": rep(att_src1), "adst1": rep(att_dst1),
        "asrc2": rep(att_src2), "adst2": rep(att_dst2),
        "bb1": rep(b1), "bb2": rep(b2),
        "iob": np.ascontiguousarray(iota.astype(BF)),
        "idn": np.ascontiguousarray(np.eye(128, dtype=np.float32)),
        "idb": np.ascontiguousarray(np.eye(128).astype(BF)),
        "kap1": kap1, "kap2": kap2,
    }

    x = np.asarray(x, np.float32)
    in_maps = []
    for c in range(n_cores):
        xs = np.ascontiguousarray(x[c * Nc:(c + 1) * Nc].T)
        in_maps.append({
            "xT": xs,
            "midx1": np.ascontiguousarray(midx1[c]),
            "midx2": np.ascontiguousarray(midx2[c]),
            "mdst": np.ascontiguousarray(mdst[c]),
            "mea": np.ascontiguousarray(mea[c]),
            **consts,
        })
    return in_maps, Nc, KCs


# ----------------------------------------------------------------------------
# public entry
# ----------------------------------------------------------------------------

_RUNNER_CACHE = {}


def _make_runner(nc, n_cores):
    """Reusable jitted shard_map executor for a Bass module (mirrors
    bass2jax.run_bass_via_pjrt but callable repeatedly)."""
    import jax
    from jax.experimental.shard_map import shard_map
    from jax.sharding import Mesh, PartitionSpec
    from concourse import bass2jax

    bass2jax.install_neuronx_cc_hook()
    partition_name = nc.partition_id_tensor.name if nc.partition_id_tensor else None

    in_names, out_names, out_avals, zshapes = [], [], [], []
    for alloc in nc.m.functions[0].allocations:
        if not isinstance(alloc, mybir.MemoryLocationSet):
            continue
        name = alloc.memorylocations[0].name
        if alloc.kind == "ExternalInput":
            if name != partition_name:
                in_names.append(name)
        elif alloc.kind == "ExternalOutput":
            shape = tuple(alloc.tensor_shape)
            dtype = mybir.dt.np(alloc.dtype)
            out_names.append(name)
            out_avals.append(jax.core.ShapedArray(shape, dtype))
            zshapes.append((shape, dtype))

    n_params, n_outs = len(in_names), len(out_names)
    all_in = list(in_names) + list(out_names)
    if partition_name is not None:
        all_in.append(partition_name)
    donate = tuple(range(n_params, n_params + n_outs))

    def _body(*args):
        operands = list(args)
        if partition_name is not None:
            operands.append(bass2jax.partition_id_tensor())
        return tuple(bass2jax._bass_exec_p.bind(
            *operands, out_avals=tuple(out_avals), in_names=tuple(all_in),
            out_names=tuple(out_names), lowering_input_output_aliases=(),
            sim_require_finite=True, sim_require_nnan=True, nc=nc))

    devices = jax.devices()[:n_cores]
    mesh = Mesh(np.asarray(devices), ("core",))
    sharded = jax.jit(
        shard_map(_body, mesh=mesh,
                  in_specs=(PartitionSpec("core"),) * (n_params + n_outs),
                  out_specs=(PartitionSpec("core"),) * n_outs,
                  check_rep=False),
        donate_argnums=donate, keep_unused=True)

    def run(in_maps):
        import jax
        concat_in = [
            np.concatenate([np.asarray(in_maps[c][n]) for c in range(n_cores)],
                           axis=0)
            for n in in_names
        ]
        zeros = [np.zeros((n_cores * s[0], *s[1:]), dt) for (s, dt) in zshapes]
        arrs = sharded(*concat_in, *zeros)
        jax.block_until_ready(arrs)
        return [
            {n: np.asarray(arrs[i]).reshape(n_cores, *out_avals[i].shape)[c]
             for i, n in enumerate(out_names)}
            for c in range(n_cores)
        ]

    return run


def prep_and_runner(**inputs):
    in_maps, Nc, KCs = host_prep(
        inputs["x"], inputs["edge_index"], inputs["edge_weight"],
        inputs["W1"], inputs["att_src1"], inputs["att_dst1"],
        inputs["W_e1"], inputs["att_e1"], inputs["b1"],
        inputs["W2"], inputs["att_src2"], inputs["att_dst2"],
        inputs["W_e2"], inputs["att_e2"], inputs["b2"],
    )
    key = (Nc, tuple(KCs))
    if key not in _RUNNER_CACHE:
        nc = build_gat(Nc, KCs)
        _RUNNER_CACHE[key] = _make_runner(nc, NCORES)
    return _RUNNER_CACHE[key], in_maps


def kernel(**inputs):
    run, in_maps = prep_and_runner(**inputs)
    res = run(in_maps)
    out = np.concatenate([res[c]["out"] for c in range(NCORES)], axis=0)
    return out.astype(np.float32)



# revision 5
# speedup vs baseline: 2.1391x; 1.8269x over previous
"""2-layer GAT (PyG GATConv semantics) on 8 Trainium2 NeuronCores.

Strategy (graph/data parallel, per sharding hint):
  - Nodes partitioned into 8 contiguous ranges (6250 per core).
  - Each core computes h = x @ W plus per-node attention scalars (a_src, a_dst)
    for its own slice, packs them into table rows [h(128) | a_s(4) | a_d(4)],
    and an AllGather replicates the full 50000x136 table to every core.
  - Edges (with self loops) are sorted by destination and assigned to the core
    owning the destination; within a core they are grouped into 49 blocks of
    128 destination slots, each padded to a uniform number of 128-edge chunks.
  - Per chunk, source rows are fetched with an indirect DMA gather (one row per
    partition); the per-destination softmax + weighted aggregation is done with
    one-hot matmuls into PSUM (segment softmax without max subtraction -- exact
    up to fp rounding since alphas are O(10)).
  - Layer 2 repeats the same edge program against the layer-2 table.

Raw Bass (no Tile -- the Tile drain is incompatible with this toolchain),
hand-scheduled with per-engine semaphore counts, 3-deep block pipelining.
One-hot matrices and messages are bf16 (values exact / tolerance-safe).
"""
import sys

sys.path.insert(0, "/opt/trn_rl_repo")

import math
from contextlib import ExitStack

import numpy as np

import concourse.bass as bass
import concourse.mybir as mybir

F32 = mybir.dt.float32
BF16 = mybir.dt.bfloat16
I32 = mybir.dt.int32
P = 128

# problem constants (hardcoded per contract)
N_NODES = 50000
N_EDGES = 800000
IN_DIM = 256
OUT_DIM = 32
HEADS = 4
HC = OUT_DIM * HEADS  # 128
NEG_SLOPE = 0.2
NCORES = 8
TW = HC + 2 * HEADS  # table row width: h | a_src | a_dst = 136
SENTINEL = 320.0     # pad-edge dst_local (bf16-exact, >= 128 so one-hot is 0)
GS = 4               # gather/message/meta pipeline depth
CHUNKS1 = (0, 2048, 4096, 6144)   # L1 table AllGather chunk row starts
CHUNKS2 = (0, 2048, 4096, 5888)   # L2 table AllGather chunk row starts
CC_OVERLAP = True
MONO_CC = False
ABLATE_GATHER = False


# ----------------------------------------------------------------------------
# Bass program builder
# ----------------------------------------------------------------------------

def build_gat(Nc, KCs, in_dim=IN_DIM):
    NB = math.ceil(Nc / 128)  # destination blocks == node tiles
    NT = NB
    KS = in_dim // P
    assert in_dim % P == 0
    assert len(KCs) == NB
    KC = max(KCs)  # array-sizing chunk count (incl. self chunk 0)

    nc = bass.Bass()

    # ---- DRAM parameters ----
    xT = nc.declare_dram_parameter("xT", [in_dim, Nc], F32, isOutput=False)
    W1 = nc.declare_dram_parameter("W1", [in_dim, HC], F32, isOutput=False)
    W2 = nc.declare_dram_parameter("W2", [HC, HC], F32, isOutput=False)
    asrc1 = nc.declare_dram_parameter("asrc1", [P, HC], F32, isOutput=False)
    adst1 = nc.declare_dram_parameter("adst1", [P, HC], F32, isOutput=False)
    asrc2 = nc.declare_dram_parameter("asrc2", [P, HC], F32, isOutput=False)
    adst2 = nc.declare_dram_parameter("adst2", [P, HC], F32, isOutput=False)
    bb1 = nc.declare_dram_parameter("bb1", [P, HC], F32, isOutput=False)
    bb2 = nc.declare_dram_parameter("bb2", [P, HC], F32, isOutput=False)
    iob = nc.declare_dram_parameter("iob", [P, P], BF16, isOutput=False)
    idn = nc.declare_dram_parameter("idn", [P, P], F32, isOutput=False)
    idb = nc.declare_dram_parameter("idb", [P, P], BF16, isOutput=False)
    kap1 = nc.declare_dram_parameter("kap1", [KC, KC * HEADS], F32, isOutput=False)
    kap2 = nc.declare_dram_parameter("kap2", [KC, KC * HEADS], F32, isOutput=False)
    midx1 = nc.declare_dram_parameter("midx1", [NB, P, KC], I32, isOutput=False)
    midx2 = nc.declare_dram_parameter("midx2", [NB, P, KC], I32, isOutput=False)
    mdst = nc.declare_dram_parameter("mdst", [NB, P, KC], F32, isOutput=False)
    mea = nc.declare_dram_parameter("mea", [NB, KC, P], F32, isOutput=False)
    out_p = nc.declare_dram_parameter("out", [Nc, HC], F32, isOutput=True)

    # ---- internal DRAM ----
    hA1s = nc.dram_tensor("hA1s", [Nc, TW], F32)
    hA2s = nc.dram_tensor("hA2s", [Nc, TW], F32)
    hA1f = nc.dram_tensor("hA1f", [Nc * NCORES, TW], F32, addr_space="Shared")
    hA2f = nc.dram_tensor("hA2f", [Nc * NCORES, TW], F32, addr_space="Shared")

    ctx = ExitStack()
    sb = lambda name, shape, dt=F32: ctx.enter_context(
        nc.sbuf_tensor(name, shape, dt))
    ps = lambda name, shape, dt=F32: ctx.enter_context(
        nc.psum_tensor(name, shape, dt))

    # ---- SBUF ----
    W1_sb = sb("W1_sb", [P, KS, P])
    W2_sb = sb("W2_sb", [P, HC])
    asrc1_sb = sb("asrc1_sb", [P, HC]); adst1_sb = sb("adst1_sb", [P, HC])
    asrc2_sb = sb("asrc2_sb", [P, HC]); adst2_sb = sb("adst2_sb", [P, HC])
    bb1_sb = sb("bb1_sb", [P, HC]); bb2_sb = sb("bb2_sb", [P, HC])
    iob_sb = sb("iob_sb", [P, P], BF16)
    idn_sb = sb("idn_sb", [P, P]); idb_sb = sb("idb_sb", [P, P], BF16)
    kap1_sb = sb("kap1_sb", [KC, KC * HEADS])
    kap2_sb = sb("kap2_sb", [KC, KC * HEADS])
    xk_sb = sb("xk_sb", [P, 2, KS, 4 * P])
    htile_sb = sb("htile_sb", [P, 2, 4, TW])
    tmp_sb = sb("tmp_sb", [P, 4, HC])
    h1_sb = sb("h1_sb", [P, NT * P])
    h1T_sb = sb("h1T_sb", [P, 2, P])
    g_sb = sb("g_sb", [P, GS, KC, TW])
    msg_sb = sb("msg_sb", [P, GS, KC, TW], BF16)
    M_sb = sb("M_sb", [P, 2, KC, P], BF16)
    Mt_sb = sb("Mt_sb", [P, 2, KC, P], BF16)
    idx_sb = sb("idx_sb", [P, GS, KC], I32)
    dst_sb = sb("dst_sb", [P, GS, KC])
    eaT_sb = sb("eaT_sb", [KC, GS, P])
    adbb_sb = sb("adbb_sb", [P, GS, HEADS], BF16)
    alpha_sb = sb("alpha_sb", [P, 2, KC, HEADS])
    alph2_sb = sb("alph2_sb", [P, KC, HEADS])
    asg_sb = sb("asg_sb", [P, 2, KC, HEADS])
    recip_sb = sb("recip_sb", [P, HEADS])
    outsb = sb("outsb", [P, 2, HC])

    # ---- PSUM (8 banks) ----
    ph = ps("ph0", [P, 4 * P])                    # h matmuls (A, D), 1 bank
    ptf = ps("ptf", [P, P])                       # f32 transposes (phase D)
    ptb = [ps("ptb0", [P, P], BF16), ps("ptb1", [P, P], BF16)]  # M transposes
    pa = [ps("pa0", [P, KC * HEADS]), ps("pa1", [P, KC * HEADS])]
    po = [ps("po0", [P, HC + HEADS]), ps("po1", [P, HC + HEADS])]

    # ---- semaphores / plan ----
    sem_names = ["s_sp", "s_pool", "s_pe", "s_dve", "s_act", "s_cc",
                 "s_spr", "s_plr", "s_g0", "s_g1", "s_g2", "s_g3"]
    sems = {n: ctx.enter_context(nc.semaphore(n)) for n in sem_names}
    ENG_SEM = {"sp": "s_sp", "pool": "s_pool", "pe": "s_pe", "dve": "s_dve",
               "act": "s_act"}
    RDY_SEM = {"sp": "s_spr", "pool": "s_plr"}

    cnt = {n: 0 for n in sem_names}
    items = {e: [] for e in ENG_SEM}
    m = {}

    def em(eng, fn=None, dma=False, cc=False, raw=False, waits=(), inc=None):
        if fn is not None and not raw:
            if inc is not None:
                pass
            elif cc:
                inc = ("s_cc", 1)
            elif dma:
                inc = (ENG_SEM[eng], 16)
            else:
                inc = (ENG_SEM[eng], 1)
            cnt[inc[0]] += inc[1]
        else:
            inc = None
        items[eng].append((fn, [(s, v) for (s, v) in waits if v > 0], inc))

    def drain(eng):
        em(eng, lambda e: e.drain(), raw=True)

    def flush(eng):
        rs = RDY_SEM[eng]
        em(eng, None, waits=[(ENG_SEM[eng], cnt[ENG_SEM[eng]])])
        em(eng, (lambda rs=rs: (lambda e: e.sem_inc(sems[rs], 1)))(), raw=True)
        cnt[rs] += 1
        return cnt[rs]

    def barrier():
        snap = dict(cnt)
        for e in ENG_SEM:
            em(e, None, waits=[(s, snap[s]) for s in sem_names])

    AL = mybir.AluOpType
    AF = mybir.ActivationFunctionType
    AX = mybir.AxisListType

    # ======================= const loads =======================
    def cdma(dst_ap, src_ap):
        em("sp", (lambda d=dst_ap, s=src_ap: (lambda e: e.dma_start(out=d, in_=s)))(),
           dma=True)

    for k in range(KS):
        cdma(W1_sb[:, k, :], W1[k * P:(k + 1) * P, :])
    cdma(W2_sb[:], W2[:])
    cdma(asrc1_sb[:], asrc1[:]); cdma(adst1_sb[:], adst1[:])
    cdma(asrc2_sb[:], asrc2[:]); cdma(adst2_sb[:], adst2[:])
    cdma(bb1_sb[:], bb1[:]); cdma(bb2_sb[:], bb2[:])
    cdma(iob_sb[:], iob[:]); cdma(idn_sb[:], idn[:]); cdma(idb_sb[:], idb[:])
    cdma(kap1_sb[:], kap1[:]); cdma(kap2_sb[:], kap2[:])
    m["constsConf"] = flush("sp")
    # self-chunk rows beyond nd of the last block are read (harmlessly) --
    # give them defined values once
    em("dve", lambda e: e.memset(g_sb[:], 0.0))

    # ======================= table phase A (batched groups of 4 tiles) ======
    GT = 4                      # tiles per group
    NG = math.ceil(Nc / (GT * P))

    def plan_tableA():
        p = "A"
        for g in range(NG):
            c0 = g * GT * P
            ng = min(GT * P, Nc - c0)
            jt = math.ceil(ng / P)          # tiles in this group
            s = g % 2
            w = [("s_pe", m.get((p, "mm", g - 2), 0))]
            for k in range(KS):
                em("sp", (lambda s=s, k=k, c0=c0, ng=ng: (lambda e: e.dma_start(
                    out=xk_sb[:, s, k, 0:ng],
                    in_=xT[k * P:(k + 1) * P, c0:c0 + ng])))(),
                   dma=True, waits=w if k == 0 else ())
            m[(p, "conf", g)] = flush("sp")
            if g >= 1:
                g1 = g - 1
                c1 = g1 * GT * P
                ng1 = min(GT * P, Nc - c1)
                jt1 = math.ceil(ng1 / P)
                if ng1 == GT * P:
                    dst_ap = hA1s[c1:c1 + ng1, :].rearrange(
                        "(j q) c -> q j c", q=P)
                    src_ap = htile_sb[:, 1 - s, :, :]
                else:
                    dst_ap = hA1s[c1:c1 + ng1, :]
                    src_ap = htile_sb[0:ng1, 1 - s, 0, :]
                em("sp", (lambda d=dst_ap, sr=src_ap: (lambda e: e.dma_start(
                    out=d, in_=sr)))(),
                   dma=True, waits=[("s_dve", m[(p, "ops", g1)])])
            w = [("s_spr", m[(p, "conf", g)]),
                 ("s_dve", m.get((p, "ops", g - 1), 0))]
            first = True
            for j in range(jt):
                nt = min(P, ng - j * P)
                for k in range(KS):
                    em("pe", (lambda s=s, k=k, j=j, nt=nt, c0=c0: (lambda e: e.matmul(
                        out=ph[0:nt, j * P:j * P + P],
                        lhsT=xk_sb[:, s, k, j * P:j * P + nt],
                        rhs=W1_sb[:, k, :], start=(k == 0), stop=(k == KS - 1),
                        skip_group_check=True)))(),
                       waits=w if first else ())
                    first = False
            m[(p, "mm", g)] = cnt["s_pe"]
            # DVE: pack table rows
            w = [("s_pe", m[(p, "mm", g)]),
                 ("s_spr", m.get((p, "conf", g), 0) if g >= 2 else 0)]
            if ng == GT * P:
                em("dve", (lambda s=s, jt=jt: (lambda e: e.tensor_copy(
                    out=htile_sb[:, s, 0:jt, 0:HC],
                    in_=ph[:, 0:jt * P].rearrange(
                        "q (j c) -> q j c", j=jt))))(), waits=w)
                m[(p, "copy", g)] = cnt["s_dve"]
                drain("dve")
                em("dve", (lambda s=s, jt=jt: (lambda e: e.tensor_mul(
                    out=tmp_sb[:, 0:jt, :],
                    in0=ph[:, 0:jt * P].rearrange("q (j c) -> q j c", j=jt),
                    in1=asrc1_sb[:, None, :].to_broadcast([P, jt, HC]))))())
                drain("dve")
                em("dve", (lambda s=s, jt=jt: (lambda e: e.tensor_reduce(
                    out=htile_sb[:, s, 0:jt, HC:HC + HEADS],
                    in_=tmp_sb[:, 0:jt, :].rearrange(
                        "q j (h c) -> q j h c", c=OUT_DIM),
                    axis=AX.X, op=AL.add)))())
                drain("dve")
                em("dve", (lambda s=s, jt=jt: (lambda e: e.tensor_mul(
                    out=tmp_sb[:, 0:jt, :],
                    in0=ph[:, 0:jt * P].rearrange("q (j c) -> q j c", j=jt),
                    in1=adst1_sb[:, None, :].to_broadcast([P, jt, HC]))))())
                drain("dve")
                em("dve", (lambda s=s, jt=jt: (lambda e: e.tensor_reduce(
                    out=htile_sb[:, s, 0:jt, HC + HEADS:TW],
                    in_=tmp_sb[:, 0:jt, :].rearrange(
                        "q j (h c) -> q j h c", c=OUT_DIM),
                    axis=AX.X, op=AL.add)))())
            else:
                first = True
                for j in range(jt):
                    nt = min(P, ng - j * P)
                    em("dve", (lambda s=s, j=j, nt=nt: (lambda e: e.tensor_copy(
                        out=htile_sb[0:nt, s, j, 0:HC],
                        in_=ph[0:nt, j * P:j * P + P])))(),
                       waits=w if first else ())
                    first = False
                    if j == 0:
                        m[(p, "copy", g)] = cnt["s_dve"]
                    drain("dve")
                    em("dve", (lambda s=s, j=j, nt=nt: (lambda e: e.tensor_mul(
                        out=tmp_sb[0:nt, 0, :],
                        in0=ph[0:nt, j * P:j * P + P],
                        in1=asrc1_sb[0:nt, :])))())
                    drain("dve")
                    em("dve", (lambda s=s, j=j, nt=nt: (lambda e: e.tensor_reduce(
                        out=htile_sb[0:nt, s, j, HC:HC + HEADS],
                        in_=tmp_sb[0:nt, 0, :].rearrange(
                            "q (h c) -> q h c", c=OUT_DIM),
                        axis=AX.X, op=AL.add)))())
                    drain("dve")
                    em("dve", (lambda s=s, j=j, nt=nt: (lambda e: e.tensor_mul(
                        out=tmp_sb[0:nt, 0, :],
                        in0=ph[0:nt, j * P:j * P + P],
                        in1=adst1_sb[0:nt, :])))())
                    drain("dve")
                    em("dve", (lambda s=s, j=j, nt=nt: (lambda e: e.tensor_reduce(
                        out=htile_sb[0:nt, s, j, HC + HEADS:TW],
                        in_=tmp_sb[0:nt, 0, :].rearrange(
                            "q (h c) -> q h c", c=OUT_DIM),
                        axis=AX.X, op=AL.add)))())
            m[(p, "ops", g)] = cnt["s_dve"]
        g = NG - 1
        c0 = g * GT * P
        ng = min(GT * P, Nc - c0)
        if ng == GT * P:
            dst_ap = hA1s[c0:c0 + ng, :].rearrange("(j q) c -> q j c", q=P)
            src_ap = htile_sb[:, g % 2, :, :]
        else:
            jt = math.ceil(ng / P)
            dst_ap = hA1s[c0:c0 + ng, :]
            src_ap = None  # handled below per-tile for ragged tail
        if src_ap is not None:
            em("sp", (lambda d=dst_ap, sr=src_ap: (lambda e: e.dma_start(
                out=d, in_=sr)))(),
               dma=True, waits=[("s_dve", m[(p, "ops", g)])])
        else:
            # ragged: store tile by tile
            w = [("s_dve", m[(p, "ops", g)])]
            for j in range(math.ceil(ng / P)):
                nt = min(P, ng - j * P)
                em("sp", (lambda s=g % 2, j=j, nt=nt, c0=c0: (lambda e: e.dma_start(
                    out=hA1s[c0 + j * P:c0 + j * P + nt, :],
                    in_=htile_sb[0:nt, s, j, :])))(),
                   dma=True, waits=w if j == 0 else ())
        m[(p, "allConf")] = flush("sp")

    # --- layer-2 table tile (merged into layer-1 edge pipeline) ---
    def plan_tableD_tile(t):
        p = "D"
        s = t % 2
        em("pe", (lambda t=t: (lambda e: e.transpose(
            out=ptf[:], in_=h1_sb[:, t * P:(t + 1) * P],
            identity=idn_sb[:])))(),
           waits=[("s_dve", m[("C", "epi", t)]),
                  ("s_dve", m.get((p, "c1", t - 1), 0))])
        m[(p, "T", t)] = cnt["s_pe"]
        em("dve", (lambda s=s: (lambda e: e.tensor_copy(
            out=h1T_sb[:, s, :], in_=ptf[:])))(),
           waits=[("s_pe", m[(p, "T", t)]),
                  ("s_pe", m.get((p, "mm", t - 2), 0))])
        m[(p, "c1", t)] = cnt["s_dve"]
        em("pe", (lambda s=s: (lambda e: e.matmul(
            out=ph[:, 0:HC], lhsT=h1T_sb[:, s, :], rhs=W2_sb[:],
            start=True, stop=True)))(),
           waits=[("s_dve", m[(p, "c1", t)]),
                  ("s_dve", m.get((p, "ops", t - 1), 0))])
        m[(p, "mm", t)] = cnt["s_pe"]
        nt = min(P, Nc - t * P)
        w = [("s_pe", m[(p, "mm", t)]),
             ("s_spr", m.get((p, "stconf", t - 2), 0))]
        em("dve", (lambda s=s: (lambda e: e.tensor_copy(
            out=htile_sb[:, s, 0, 0:HC], in_=ph[:, 0:HC])))(), waits=w)
        m[(p, "copy", t)] = cnt["s_dve"]
        drain("dve")
        em("dve", (lambda s=s: (lambda e: e.tensor_mul(
            out=tmp_sb[:, 0, :], in0=ph[:, 0:HC],
            in1=asrc2_sb[:])))())
        drain("dve")
        em("dve", (lambda s=s: (lambda e: e.tensor_reduce(
            out=htile_sb[:, s, 0, HC:HC + HEADS],
            in_=tmp_sb[:, 0, :].rearrange("q (h c) -> q h c", c=OUT_DIM),
            axis=AX.X, op=AL.add)))())
        drain("dve")
        em("dve", (lambda s=s: (lambda e: e.tensor_mul(
            out=tmp_sb[:, 0, :], in0=ph[:, 0:HC],
            in1=adst2_sb[:])))())
        drain("dve")
        em("dve", (lambda s=s: (lambda e: e.tensor_reduce(
            out=htile_sb[:, s, 0, HC + HEADS:TW],
            in_=tmp_sb[:, 0, :].rearrange("q (h c) -> q h c", c=OUT_DIM),
            axis=AX.X, op=AL.add)))())
        m[(p, "ops", t)] = cnt["s_dve"]

    # ======================= edge phase (C: layer1, F: layer2) ===============
    # Software-pipelined: block b's gathers are issued in loop iter b, its
    # compute consumption happens in iter b+1, and the Pool confirmation flush
    # runs only every second block so the DMA ring never fully drains per
    # block. Chunk 0 of each block is the self-loop chunk (regular DMA,
    # identity one-hot, doubles as a_dst source + denominator guarantee).
    def plan_edges(layer):
        p = "C" if layer == 1 else "F"
        hAf = hA1f if layer == 1 else hA2f
        hAs = hA1s if layer == 1 else hA2s
        midx = midx1 if layer == 1 else midx2
        kap_sb = kap1_sb if layer == 1 else kap2_sb
        bias_sb = bb1_sb if layer == 1 else bb2_sb

        def plan_meta(b):
            nd = min(P, Nc - b * P)
            s = b % GS
            w = [(f"s_g{b % GS}", m.get((p, "gcnt", b - GS), 0)),
                 ("s_pe", m.get((p, "scat", b - GS), 0))]
            em("sp", (lambda s=s, b=b: (lambda e: e.dma_start(
                out=idx_sb[:, s, :], in_=midx[b])))(), dma=True, waits=w)
            em("sp", (lambda s=s, b=b: (lambda e: e.dma_start(
                out=dst_sb[:, s, :], in_=mdst[b])))(), dma=True)
            em("sp", (lambda s=s, b=b: (lambda e: e.dma_start(
                out=eaT_sb[:, s, :], in_=mea[b])))(), dma=True)
            em("sp", (lambda s=s, b=b, nd=nd, hAs=hAs: (lambda e: e.dma_start(
                out=g_sb[0:nd, s, 0, :],
                in_=hAs[b * P:b * P + nd, :])))(), dma=True)
            if layer == 1 and b >= 2:
                t1 = b - 2
                nt1 = min(P, Nc - t1 * P)
                em("sp", (lambda s=t1 % 2, t1=t1, nt1=nt1: (lambda e: e.dma_start(
                    out=hA2s[t1 * P:t1 * P + nt1, :],
                    in_=htile_sb[0:nt1, s, 0, :])))(),
                   dma=True, waits=[("s_dve", m[("D", "ops", t1)])])
            if layer == 2 and b >= 2:
                b1 = b - 2
                nd1 = min(P, Nc - b1 * P)
                em("sp", (lambda sp=b1 % 2, b1=b1, nd1=nd1: (lambda e: e.dma_start(
                    out=out_p[b1 * P:b1 * P + nd1, :],
                    in_=outsb[0:nd1, sp, :])))(),
                   dma=True, waits=[("s_dve", m[(p, "epi", b1)])])
            m[(p, "conf", b)] = flush("sp")
            if layer == 1 and b >= 2:
                m[("D", "stconf", b - 2)] = m[(p, "conf", b)]

        def plan_gather(b):
            KCb = KCs[b]
            s = b % GS
            sg = f"s_g{s}"
            w = [("s_spr", m[(p, "conf", b)]),
                 ("s_dve", m.get((p, "msgs", b - GS), 0))]
            for c in range(1, KCb):
                if ABLATE_GATHER:
                    break
                em("pool", (lambda s=s, c=c, hAf=hAf: (lambda e: e.indirect_dma_start(
                    out=g_sb[:, s, c, :], out_offset=None, in_=hAf[:],
                    in_offset=bass.IndirectOffsetOnAxis(
                        ap=idx_sb[:, s, c:c + 1], axis=0))))(),
                   dma=True, waits=w if c == 1 else (), inc=(sg, 16))
            m[(p, "gcnt", b)] = cnt[sg]

        def plan_consume(b):
            KCb = KCs[b]
            s = b % GS
            s2 = b % 2
            # DVE: adb cast + one-hot builds
            w = [("s_spr", m[(p, "conf", b)]),
                 ("s_pe", m.get((p, "scat", b - 2), 0))]
            em("dve", (lambda s=s: (lambda e: e.tensor_copy(
                out=adbb_sb[:, s, :], in_=g_sb[:, s, 0, HC + HEADS:TW])))(),
               waits=w)
            for c in range(1, KCb):
                em("dve", (lambda s=s, s2=s2, c=c: (lambda e: e.tensor_scalar(
                    out=M_sb[:, s2, c, :], in0=iob_sb[:],
                    scalar1=dst_sb[:, s, c:c + 1], scalar2=None,
                    op0=AL.is_equal)))())
                m[(p, "Mb", b, c)] = cnt["s_dve"]
            for c in range(1, KCb):
                em("pe", (lambda s2=s2, c=c: (lambda e: e.transpose(
                    out=ptb[c % 2][:], in_=M_sb[:, s2, c, :],
                    identity=idb_sb[:])))(),
                   waits=[("s_dve", m[(p, "Mb", b, c)]),
                          ("s_dve", m.get((p, "Mtc", b, c - 2),
                                          m.get((p, "MtcPrev", b), 0)))])
                m[(p, "T", b, c)] = cnt["s_pe"]
                em("dve", (lambda s2=s2, c=c: (lambda e: e.tensor_copy(
                    out=Mt_sb[:, s2, c, :], in_=ptb[c % 2][:])))(),
                   waits=[("s_pe", m[(p, "T", b, c)])])
                m[(p, "Mtc", b, c)] = cnt["s_dve"]
            m[(p, "MtcPrev", b + 1)] = m[(p, "Mtc", b, KCb - 1)]
            # PE: alpha accumulation
            em("pe", (lambda s=s, s2=s2, KCb=KCb, kap_sb=kap_sb: (lambda e: e.matmul(
                out=pa[s2][:, 0:KCb * HEADS],
                lhsT=eaT_sb[0:KCb, s, :], rhs=kap_sb[0:KCb, 0:KCb * HEADS],
                start=True, stop=False, skip_group_check=True)))(),
               waits=[("s_spr", m[(p, "conf", b)]),
                      ("s_dve", m.get((p, "alpha", b - 2), 0))])
            em("pe", (lambda s=s, s2=s2: (lambda e: e.matmul(
                out=pa[s2][:, 0:HEADS], lhsT=idb_sb[:], rhs=adbb_sb[:, s, :],
                start=False, stop=False, skip_group_check=True)))(),
               waits=[("s_dve", m[(p, "MtcPrev", b + 1)])])
            for c in range(1, KCb):
                em("pe", (lambda s=s, s2=s2, c=c, KCb=KCb: (lambda e: e.matmul(
                    out=pa[s2][:, c * HEADS:(c + 1) * HEADS],
                    lhsT=Mt_sb[:, s2, c, :], rhs=adbb_sb[:, s, :],
                    start=False, stop=(c == KCb - 1), skip_group_check=True)))())
            m[(p, "admm", b)] = cnt["s_pe"]
            # DVE: alpha + leaky relu
            em("dve", (lambda s=s, s2=s2, KCb=KCb: (lambda e: e.tensor_add(
                out=alpha_sb[:, s2, 0:KCb, :],
                in0=g_sb[:, s, 0:KCb, HC:HC + HEADS],
                in1=pa[s2][:, 0:KCb * HEADS].rearrange(
                    "p (k h) -> p k h", h=HEADS))))(),
               waits=[("s_pe", m[(p, "admm", b)]),
                      (f"s_g{b % GS}", m[(p, "gcnt", b)])])
            drain("dve")
            em("dve", (lambda s2=s2, KCb=KCb: (lambda e: e.tensor_scalar(
                out=alph2_sb[:, 0:KCb, :], in0=alpha_sb[:, s2, 0:KCb, :],
                scalar1=NEG_SLOPE, scalar2=None, op0=AL.mult)))())
            drain("dve")
            em("dve", (lambda s2=s2, KCb=KCb: (lambda e: e.tensor_tensor(
                out=alpha_sb[:, s2, 0:KCb, :], in0=alpha_sb[:, s2, 0:KCb, :],
                in1=alph2_sb[:, 0:KCb, :], op=AL.max)))())
            m[(p, "alpha", b)] = cnt["s_dve"]
            # ACT: exp
            em("act", (lambda s=s, s2=s2, KCb=KCb: (lambda e: e.activation(
                out=msg_sb[:, s, 0:KCb, HC:HC + HEADS],
                in_=alpha_sb[:, s2, 0:KCb, :], func=AF.Exp)))(),
               waits=[("s_dve", m[(p, "alpha", b)]),
                      ("s_pe", m.get((p, "scat", b - GS), 0))])
            m[(p, "exp", b)] = cnt["s_act"]
            # DVE: messages
            em("dve", (lambda s=s, KCb=KCb: (lambda e: e.tensor_mul(
                out=msg_sb[:, s, 0:KCb, 0:HC].rearrange(
                    "p k (h c) -> p k h c", c=OUT_DIM),
                in0=g_sb[:, s, 0:KCb, 0:HC].rearrange(
                    "p k (h c) -> p k h c", c=OUT_DIM),
                in1=msg_sb[:, s, 0:KCb, HC:HC + HEADS][:, :, :, None].to_broadcast(
                    [P, KCb, HEADS, OUT_DIM]))))(),
               waits=[("s_act", m[(p, "exp", b)])])
            m[(p, "msgs", b)] = cnt["s_dve"]
            # PE: scatter
            w = [("s_dve", m[(p, "msgs", b)]),
                 ("s_dve", m.get((p, "epi", b - 2), 0))]
            em("pe", (lambda s=s, s2=s2: (lambda e: e.matmul(
                out=po[s2][:], lhsT=idb_sb[:],
                rhs=msg_sb[:, s, 0, 0:HC + HEADS],
                start=True, stop=False)))(), waits=w)
            for c in range(1, KCb):
                em("pe", (lambda s=s, s2=s2, c=c, KCb=KCb: (lambda e: e.matmul(
                    out=po[s2][:], lhsT=M_sb[:, s2, c, :],
                    rhs=msg_sb[:, s, c, 0:HC + HEADS],
                    start=False, stop=(c == KCb - 1))))())
            m[(p, "scat", b)] = cnt["s_pe"]
            # DVE: epilogue
            w = [("s_pe", m[(p, "scat", b)])]
            if layer == 2:
                w.append(("s_spr", m[(p, "conf", b)]))
            em("dve", (lambda s2=s2: (lambda e: e.reciprocal(
                out=recip_sb[:], in_=po[s2][:, HC:HC + HEADS])))(), waits=w)
            if layer == 1:
                tgt = lambda b=b: h1_sb[:, b * P:(b + 1) * P]
            else:
                tgt = lambda s2=s2: outsb[:, s2, :]
            drain("dve")
            em("dve", (lambda s2=s2, tgt=tgt: (lambda e: e.tensor_mul(
                out=tgt().rearrange("p (h c) -> p h c", c=OUT_DIM),
                in0=po[s2][:, 0:HC].rearrange("p (h c) -> p h c", c=OUT_DIM),
                in1=recip_sb[:][:, :, None].to_broadcast(
                    [P, HEADS, OUT_DIM]))))())
            drain("dve")
            em("dve", (lambda tgt=tgt, bias_sb=bias_sb: (lambda e: e.tensor_add(
                out=tgt(), in0=tgt(), in1=bias_sb[:])))())
            m[(p, "epi", b)] = cnt["s_dve"]

        l2cc = {}
        for b in range(NB + 1):
            if b < NB:
                plan_meta(b)
                plan_gather(b)
                if b in l2cc:
                    r0, r1 = l2cc[b]
                    plan_cc_chunk(hA2s, hA2f, r0, r1, m[(p, "conf", b)])
            if b >= 1:
                plan_consume(b - 1)
                if layer == 1:
                    plan_tableD_tile(b - 1)
        # tails
        if layer == 1:
            for t1 in (NB - 2, NB - 1):
                nt1 = min(P, Nc - t1 * P)
                em("sp", (lambda s=t1 % 2, t1=t1, nt1=nt1: (lambda e: e.dma_start(
                    out=hA2s[t1 * P:t1 * P + nt1, :],
                    in_=htile_sb[0:nt1, s, 0, :])))(),
                   dma=True, waits=[("s_dve", m[("D", "ops", t1)])])
            m[("D", "allConf")] = flush("sp")
            plan_cc_chunk(hA2s, hA2f, 0, Nc, m[("D", "allConf")])
        if layer == 2:
            for b1 in (NB - 2, NB - 1):
                nd1 = min(P, Nc - b1 * P)
                em("sp", (lambda sp=b1 % 2, b1=b1, nd1=nd1: (lambda e: e.dma_start(
                    out=out_p[b1 * P:b1 * P + nd1, :],
                    in_=outsb[0:nd1, sp, :])))(),
                   dma=True, waits=[("s_dve", m[(p, "epi", b1)])])
            m[(p, "allConf")] = flush("sp")

    def plan_cc_chunk(hAs, hAf, r0, r1, conf_val):
        em("pool", (lambda hAs=hAs, hAf=hAf, r0=r0, r1=r1: (lambda e: e.collective_compute(
            "AllGather", mybir.AluOpType.bypass,
            replica_groups=[list(range(NCORES))],
            ins=[hAs[r0:r1, :]],
            outs=[hAf[NCORES * r0:NCORES * r1, :]])))(),
           cc=True, waits=[("s_spr", conf_val)])

    # ======================= assemble ========================================
    plan_tableA()
    # L1 table AllGather in 4 chunks, dispatched as quarters of hA1s land
    GROUP_ROWS = GT * P
    if MONO_CC:
        plan_cc_chunk(hA1s, hA1f, 0, Nc, m[("A", "allConf")])
    else:
        for q in range(3):
            r0, r1 = CHUNKS1[q], CHUNKS1[q + 1]
            qg = r1 // GROUP_ROWS
            cv = m[("A", "conf", qg)] if CC_OVERLAP else m[("A", "allConf")]
            plan_cc_chunk(hA1s, hA1f, r0, r1, cv)
        plan_cc_chunk(hA1s, hA1f, CHUNKS1[3], Nc, m[("A", "allConf")])
    barrier()
    plan_edges(1)
    barrier()
    plan_edges(2)
    barrier()

    # ======================= emit ============================================
    lowp = nc.allow_low_precision(reason="bf16 table rows: tolerance 2e-2")
    lowp.__enter__()
    with nc.Block() as block:
        def emit_for(eng_name):
            def runner(eng):
                hwm = {n: 0 for n in sem_names}
                for fn, waits, inc in items[eng_name]:
                    for sname, v in waits:
                        if v > hwm[sname]:
                            eng.wait_ge(sems[sname], v)
                            hwm[sname] = v
                    if fn is not None:
                        inst = fn(eng)
                        if inc is not None:
                            inst.then_inc(sems[inc[0]], inc[1])
            return runner

        block.sync(emit_for("sp"))
        block.gpsimd(emit_for("pool"))
        block.tensor(emit_for("pe"))
        block.vector(emit_for("dve"))
        block.scalar(emit_for("act"))

    lowp.__exit__(None, None, None)
    ctx.close()
    return nc


# ----------------------------------------------------------------------------
# Host-side preparation
# ----------------------------------------------------------------------------

def host_prep(x, edge_index, edge_weight,
              W1, att_src1, att_dst1, W_e1, att_e1, b1,
              W2, att_src2, att_dst2, W_e2, att_e2, b2,
              n_cores=NCORES):
    import ml_dtypes
    BF = ml_dtypes.bfloat16

    N = x.shape[0]
    Nc = N // n_cores
    NB = math.ceil(Nc / 128)

    src0 = np.asarray(edge_index[0], dtype=np.int64)
    dst0 = np.asarray(edge_index[1], dtype=np.int64)
    ew = np.asarray(edge_weight, dtype=np.float32)
    ea_mean = float(ew.mean())
    # self loops are NOT in the stream: chunk 0 of each block handles them
    order = np.argsort(dst0, kind="stable")
    src, dst, ea = src0[order], dst0[order], ew[order]

    core = dst // Nc
    local = dst - core * Nc
    blk = np.minimum(local // 128, NB - 1)
    gid = core * NB + blk
    counts = np.bincount(gid, minlength=n_cores * NB)

    # per-block-index gather chunk count (max over cores), +1 for self chunk
    cpb = counts.reshape(n_cores, NB)
    KCs = [1 + int(np.ceil(cpb[:, b].max() / 128.0)) for b in range(NB)]
    KC = max(max(KCs), 2)

    gstart = np.zeros(n_cores * NB + 1, dtype=np.int64)
    np.cumsum(counts, out=gstart[1:])
    pos = np.arange(src.shape[0], dtype=np.int64) - gstart[gid]

    mdst = np.full((n_cores, NB, 128, KC), SENTINEL, dtype=np.float32)
    mea = np.zeros((n_cores, NB, KC, 128), dtype=np.float32)
    mea[:, :, 0, :] = ea_mean   # self-loop edge attr

    pp = (pos % 128).astype(np.int64)
    cc = 1 + (pos // 128).astype(np.int64)   # gather chunks start at 1
    mdst[core, blk, pp, cc] = (local - blk * 128).astype(np.float32)
    mea[core, blk, cc, pp] = ea

    # gathered-table row index under chunk-major AllGather layout:
    # row(g) = 8*B[q] + srccore*(B[q+1]-B[q]) + (l - B[q]),  l = g % Nc in
    # chunk q of boundaries B.
    def chunked_rows(g, bounds):
        B = np.asarray(list(bounds) + [Nc], dtype=np.int64)
        sc = g // Nc
        l = g % Nc
        q = np.searchsorted(B, l, side="right") - 1
        return (n_cores * B[q] + sc * (B[q + 1] - B[q]) + (l - B[q])).astype(
            np.int32)

    b1_ = (0,) if MONO_CC else CHUNKS1
    b2_ = (0,)
    midx1 = np.zeros((n_cores, NB, 128, KC), dtype=np.int32)
    midx2 = np.zeros((n_cores, NB, 128, KC), dtype=np.int32)
    midx1[core, blk, pp, cc] = chunked_rows(src, b1_)
    midx2[core, blk, pp, cc] = chunked_rows(src, b2_)

    W1 = np.asarray(W1, np.float32)
    W2 = np.asarray(W2, np.float32)
    kr1 = (np.asarray(W_e1, np.float32).reshape(HEADS, OUT_DIM)
           * np.asarray(att_e1, np.float32)).sum(1)
    kr2 = (np.asarray(W_e2, np.float32).reshape(HEADS, OUT_DIM)
           * np.asarray(att_e2, np.float32)).sum(1)
    kap1 = np.zeros((KC, KC * HEADS), np.float32)
    kap2 = np.zeros((KC, KC * HEADS), np.float32)
    for c in range(KC):
        kap1[c, c * HEADS:(c + 1) * HEADS] = kr1
        kap2[c, c * HEADS:(c + 1) * HEADS] = kr2

    rep = lambda v: np.ascontiguousarray(
        np.tile(np.asarray(v, np.float32).reshape(1, HC), (128, 1)))
    iota = np.tile(np.arange(128, dtype=np.float32), (128, 1))
    consts = {
        "W1": np.ascontiguousarray(W1),
        "W2": np.ascontiguousarray(W2),
        "asrc1": rep(att_src1), "adst1": rep(att_dst1),
        "asrc2": rep(att_src2), "adst2": rep(att_dst2),
        "bb1": rep(b1), "bb2": rep(b2),
        "iob": np.ascontiguousarray(iota.astype(BF)),
        "idn": np.ascontiguousarray(np.eye(128, dtype=np.float32)),
        "idb": np.ascontiguousarray(np.eye(128).astype(BF)),
        "kap1": kap1, "kap2": kap2,
    }

    x = np.asarray(x, np.float32)
    in_maps = []
    for c in range(n_cores):
        xs = np.ascontiguousarray(x[c * Nc:(c + 1) * Nc].T)
        in_maps.append({
            "xT": xs,
            "midx1": np.ascontiguousarray(midx1[c]),
            "midx2": np.ascontiguousarray(midx2[c]),
            "mdst": np.ascontiguousarray(mdst[c]),
            "mea": np.ascontiguousarray(mea[c]),
            **consts,
        })
    return in_maps, Nc, KCs


# ----------------------------------------------------------------------------
# public entry
# ----------------------------------------------------------------------------

_RUNNER_CACHE = {}


def _make_runner(nc, n_cores):
    """Reusable jitted shard_map executor for a Bass module (mirrors
    bass2jax.run_bass_via_pjrt but callable repeatedly)."""
    import jax
    from jax.experimental.shard_map import shard_map
    from jax.sharding import Mesh, PartitionSpec
    from concourse import bass2jax

    bass2jax.install_neuronx_cc_hook()
    partition_name = nc.partition_id_tensor.name if nc.partition_id_tensor else None

    in_names, out_names, out_avals, zshapes = [], [], [], []
    for alloc in nc.m.functions[0].allocations:
        if not isinstance(alloc, mybir.MemoryLocationSet):
            continue
        name = alloc.memorylocations[0].name
        if alloc.kind == "ExternalInput":
            if name != partition_name:
                in_names.append(name)
        elif alloc.kind == "ExternalOutput":
            shape = tuple(alloc.tensor_shape)
            dtype = mybir.dt.np(alloc.dtype)
            out_names.append(name)
            out_avals.append(jax.core.ShapedArray(shape, dtype))
            zshapes.append((shape, dtype))

    n_params, n_outs = len(in_names), len(out_names)
    all_in = list(in_names) + list(out_names)
    if partition_name is not None:
        all_in.append(partition_name)
    donate = tuple(range(n_params, n_params + n_outs))

    def _body(*args):
        operands = list(args)
        if partition_name is not None:
            operands.append(bass2jax.partition_id_tensor())
        return tuple(bass2jax._bass_exec_p.bind(
            *operands, out_avals=tuple(out_avals), in_names=tuple(all_in),
            out_names=tuple(out_names), lowering_input_output_aliases=(),
            sim_require_finite=True, sim_require_nnan=True, nc=nc))

    devices = jax.devices()[:n_cores]
    mesh = Mesh(np.asarray(devices), ("core",))
    sharded = jax.jit(
        shard_map(_body, mesh=mesh,
                  in_specs=(PartitionSpec("core"),) * (n_params + n_outs),
                  out_specs=(PartitionSpec("core"),) * n_outs,
                  check_rep=False),
        donate_argnums=donate, keep_unused=True)

    def run(in_maps):
        import jax
        concat_in = [
            np.concatenate([np.asarray(in_maps[c][n]) for c in range(n_cores)],
                           axis=0)
            for n in in_names
        ]
        zeros = [np.zeros((n_cores * s[0], *s[1:]), dt) for (s, dt) in zshapes]
        arrs = sharded(*concat_in, *zeros)
        jax.block_until_ready(arrs)
        return [
            {n: np.asarray(arrs[i]).reshape(n_cores, *out_avals[i].shape)[c]
             for i, n in enumerate(out_names)}
            for c in range(n_cores)
        ]

    return run


def prep_and_runner(**inputs):
    in_maps, Nc, KCs = host_prep(
        inputs["x"], inputs["edge_index"], inputs["edge_weight"],
        inputs["W1"], inputs["att_src1"], inputs["att_dst1"],
        inputs["W_e1"], inputs["att_e1"], inputs["b1"],
        inputs["W2"], inputs["att_src2"], inputs["att_dst2"],
        inputs["W_e2"], inputs["att_e2"], inputs["b2"],
    )
    key = (Nc, tuple(KCs))
    if key not in _RUNNER_CACHE:
        nc = build_gat(Nc, KCs)
        _RUNNER_CACHE[key] = _make_runner(nc, NCORES)
    return _RUNNER_CACHE[key], in_maps


def kernel(**inputs):
    run, in_maps = prep_and_runner(**inputs)
    res = run(in_maps)
    out = np.concatenate([res[c]["out"] for c in range(NCORES)], axis=0)
    return out.astype(np.float32)

